# revision 1
# baseline (speedup 1.0000x reference)
"""Trainium2 Bass kernel for nn_Block_54219667145535 (linear-attention block).

Sharding: 8 cores, 2 per batch (B=4). Each core computes the full-batch
k/v projection + [D,D] kv state (duplicated within the pair -> no
cross-core communication), and q/attention/FFN for its own half of the
sequence (2048 tokens). Single SPMD launch; matmuls in float32r.
"""

import os
import sys
from contextlib import ExitStack

import numpy as np


def _ensure_paths():
    for p in ("/opt/trn_rl_repo", "/root/.axon_site/_ro/trn_rl_repo"):
        if os.path.isdir(p) and p not in sys.path:
            sys.path.insert(0, p)
    try:
        import concourse.bass  # noqa: F401
    except ImportError as e:  # pragma: no cover
        raise ImportError(f"concourse not importable: {e}")


_ensure_paths()

import concourse.bass as bass  # noqa: E402
import concourse.bacc as bacc  # noqa: E402
import concourse.tile as tile  # noqa: E402
from concourse import mybir  # noqa: E402
from concourse.bass import ts  # noqa: E402
from concourse.masks import make_identity  # noqa: E402
from concourse import bass_isa  # noqa: E402

F32 = mybir.dt.float32
F32R = mybir.dt.float32r
AF = mybir.ActivationFunctionType
ALU = mybir.AluOpType
AX = mybir.AxisListType

D = 1024
DCH = 8  # d chunks of 128
H_PAD = 2816
HCH = 22  # h chunks of 128
LN_EPS = 1e-5
ATTN_EPS = 1e-6

MM_DT = F32R  # matmul streaming dtype


def _r(ap):
    """Matmul-feeding tiles are already float32r; pass through."""
    return ap


def _bcast_row(nc, row_ap, parts=128):
    """AP that reads a [1, N] DRAM row replicated across `parts` partitions."""
    return bass.AP(
        tensor=row_ap.tensor,
        offset=row_ap.offset,
        ap=[[0, parts]] + [list(d) for d in row_ap.ap[1:]],
    )


def _scatter_row(row_ap, p, c):
    """View a [1, p*c] SBUF row as [1, p, c] with p-fastest order (for DMA
    partition-scatter: out[p, c] = row[c*p_count + p] -> in dims (p, c))."""
    return bass.AP(
        tensor=row_ap.tensor,
        offset=row_ap.offset,
        ap=[list(row_ap.ap[0]), [1, p], [p, c]],
    )


def build_program(T_OWN=2048, n_cores=8):
    """Build the per-core Bass/Tile program. Returns (nc, input_names).

    Each pair of cores (2b, 2b+1) handles batch b; k/v+kv state is computed
    on own tokens only and pair-AllReduced (bf16) before attention."""
    assert T_OWN % 512 == 0
    NBLK = T_OWN // 512  # P1 blocks (own tokens only)
    NTG = T_OWN // 512  # P2 tgroups
    GROUPS = [[c, c + 1] for c in range(0, n_cores, 2)]

    nc = bacc.Bacc(
        "TRN2",
        target_bir_lowering=False,
        debug=False,
        enable_asserts=False,
        num_devices=8,
        num_swdge_queues=4,
    )

    # ---- I/O ----
    x_ownT = nc.dram_tensor("x_ownT", [D, T_OWN], F32R, kind="ExternalInput").ap()
    x_own = nc.dram_tensor("x_own", [T_OWN, D], F32, kind="ExternalInput").ap()
    wq = nc.dram_tensor("wq", [D, D], F32R, kind="ExternalInput").ap()
    wk = nc.dram_tensor("wk", [D, D], F32R, kind="ExternalInput").ap()
    wv = nc.dram_tensor("wv", [D, D], F32R, kind="ExternalInput").ap()
    bq_pre = nc.dram_tensor("bq_pre", [128, DCH], F32, kind="ExternalInput").ap()
    bk_row = nc.dram_tensor("bk_row", [1, D], F32, kind="ExternalInput").ap()
    bv_row = nc.dram_tensor("bv_row", [1, D], F32, kind="ExternalInput").ap()
    wg = nc.dram_tensor("wg", [D, H_PAD], F32R, kind="ExternalInput").ap()
    wu = nc.dram_tensor("wu", [D, H_PAD], F32R, kind="ExternalInput").ap()
    bg_pre = nc.dram_tensor("bg_pre", [128, HCH], F32, kind="ExternalInput").ap()
    bu_pre = nc.dram_tensor("bu_pre", [128, HCH], F32, kind="ExternalInput").ap()
    wd = nc.dram_tensor("wd", [H_PAD, D], F32R, kind="ExternalInput").ap()
    bd_row = nc.dram_tensor("bd_row", [1, D], F32, kind="ExternalInput").ap()
    g1_row = nc.dram_tensor("g1_row", [1, D], F32, kind="ExternalInput").ap()
    b1_row = nc.dram_tensor("b1_row", [1, D], F32, kind="ExternalInput").ap()
    g2_row = nc.dram_tensor("g2_row", [1, D], F32, kind="ExternalInput").ap()
    b2_row = nc.dram_tensor("b2_row", [1, D], F32, kind="ExternalInput").ap()
    ones2 = nc.dram_tensor("ones2", [128, 2], F32R, kind="ExternalInput").ap()
    out = nc.dram_tensor("out", [T_OWN, D], F32, kind="ExternalOutput").ap()

    input_names = [
        "x_ownT", "x_own", "wq", "wk", "wv", "bq_pre", "bk_row",
        "bv_row", "wg", "wu", "bg_pre", "bu_pre", "wd", "bd_row",
        "g1_row", "b1_row", "g2_row", "b2_row", "ones2",
    ]

    # d-chunked views of DRAM (partition-inner): [(c p) t -> p c t]
    x_ownT_v = x_ownT.rearrange("(c p) t -> p c t", p=128)
    wq_v = wq.rearrange("(c p) n -> p c n", p=128)
    wk_v = wk.rearrange("(c p) n -> p c n", p=128)
    wv_v = wv.rearrange("(c p) n -> p c n", p=128)
    wg_v = wg.rearrange("(c p) n -> p c n", p=128)
    wu_v = wu.rearrange("(c p) n -> p c n", p=128)

    with tile.TileContext(nc) as tc, ExitStack() as top:
        dram = top.enter_context(tc.tile_pool(name="dram", bufs=1, space="DRAM"))
        x1_dram = dram.tile([T_OWN, D], F32, name="x1_dram")
        x1T_dram = dram.tile([D, T_OWN], F32R, name="x1T_dram")
        x1T_dram_v = x1T_dram[:].rearrange("(c p) t -> p c t", p=128)

        consts = top.enter_context(tc.tile_pool(name="consts", bufs=1))
        ident = consts.tile([128, 128], F32, name="ident")
        make_identity(nc, ident[:])
        epsb = consts.tile([128, 1], F32, name="epsb")
        nc.vector.memset(epsb[:], LN_EPS)
        bq_s = consts.tile([128, DCH], F32, name="bq_s")
        nc.sync.dma_start(out=bq_s[:], in_=bq_pre)
        bg_s = consts.tile([128, HCH], F32, name="bg_s")
        nc.sync.dma_start(out=bg_s[:], in_=bg_pre)
        bu_s = consts.tile([128, HCH], F32, name="bu_s")
        nc.sync.dma_start(out=bu_s[:], in_=bu_pre)

        # kv state accumulators (live P1..P2 only)
        p12 = top.enter_context(ExitStack())
        accs = p12.enter_context(tc.tile_pool(name="accs", bufs=1))
        BF16 = mybir.dt.bfloat16
        kv_acc = accs.tile([128, DCH, D], BF16, name="kv_acc")  # 16KB/part
        ksum_acc = accs.tile([128, DCH, 2], BF16, name="ksum_acc")
        ones2_t = accs.tile([128, 2], F32R, name="ones2_t")
        nc.sync.dma_start(out=ones2_t[:], in_=ones2)

        # ---------------- P1: k/v projection + kv/ksum over full batch ----
        with ExitStack() as p1:
            c1_p = p1.enter_context(tc.tile_pool(name="c1", bufs=1))
            wkv_p = p1.enter_context(tc.tile_pool(name="wkv", bufs=1))
            xb_p = p1.enter_context(tc.tile_pool(name="xb", bufs=2))
            kpv_p = p1.enter_context(tc.tile_pool(name="kpv", bufs=1))
            tmp_p = p1.enter_context(tc.tile_pool(name="p1tmp", bufs=3))
            ks_p = p1.enter_context(tc.tile_pool(name="ksr", bufs=1))
            ps_proj = p1.enter_context(
                tc.tile_pool(name="ps_proj", bufs=3, space="PSUM"))
            ps_ks = p1.enter_context(
                tc.tile_pool(name="ps_ks", bufs=2, space="PSUM"))
            ps_kv = p1.enter_context(
                tc.tile_pool(name="ps_kv", bufs=3, space="PSUM"))
            ks_ps = [ps_ks.tile([2, 512], F32, name=f"ksps{e}", tag="ps_ks")
                     for e in range(2)]

            # first x block before weights so PE can start ASAP
            xblk0 = xb_p.tile([128, DCH, 512], F32R, name="xblk0", tag="xblk")
            for t4 in range(4):
                nc.sync.dma_start(out=xblk0[:, :, ts(t4, 128)],
                                  in_=x_ownT_v[:, :, ts(t4, 128)])
            wh = {}
            for which, half in ((0, 0), (1, 0), (0, 1), (1, 1)):
                w_v = wk_v if which == 0 else wv_v
                nm = f"w{'k' if which == 0 else 'v'}h{half}"
                t = wkv_p.tile([128, DCH, 512], F32R, name=nm)
                for dc in range(DCH):
                    nc.scalar.dma_start(
                        out=t[:, dc, :],
                        in_=w_v[:, dc, ts(half, 512)])
                wh[(which, half)] = t
            bkb = c1_p.tile([128, D], F32, name="bkb")
            nc.sync.dma_start(out=bkb[:], in_=_bcast_row(nc, bk_row))
            bvb = c1_p.tile([128, D], F32, name="bvb")
            nc.sync.dma_start(out=bvb[:], in_=_bcast_row(nc, bv_row))

            for blk in range(NBLK):
                if blk == 0:
                    xblk = xblk0
                else:
                    xblk = xb_p.tile([128, DCH, 512], F32R, name=f"xblk{blk}",
                                     tag="xblk")
                    nc.sync.dma_start(out=xblk[:],
                                      in_=x_ownT_v[:, :, ts(blk, 512)])

                kp_blk = kpv_p.tile([128, 4, D], F32R, name=f"kp{blk}", tag="kp")
                v_blk = kpv_p.tile([128, 4, D], F32R, name=f"v{blk}", tag="v")

                for t4 in range(4):
                    for which, half in ((0, 0), (1, 0), (0, 1), (1, 1)):
                        w_s = wh[(which, half)]
                        gsl = ts(half, 512)
                        ps = ps_proj.tile([128, 512], F32,
                                          name=f"pp{blk}_{t4}_{which}_{half}",
                                          tag="ps_proj")
                        for dc in range(DCH):
                            nc.tensor.matmul(
                                ps[:], xblk[:, dc, ts(t4, 128)],
                                w_s[:, dc, :],
                                start=(dc == 0), stop=(dc == DCH - 1))
                        if which == 0:
                            # kp = relu(k+bk) + exp(min(k+bk, 0))
                            kb = tmp_p.tile([128, 512], F32,
                                            name=f"kb{blk}_{t4}_{half}", tag="kb")
                            nc.vector.tensor_tensor(
                                out=kb[:], in0=ps[:], in1=bkb[:, gsl], op=ALU.add)
                            rl = tmp_p.tile([128, 512], F32,
                                            name=f"rl{blk}_{t4}_{half}", tag="rl")
                            nc.scalar.activation(rl[:], kb[:], AF.Relu)
                            nc.vector.tensor_tensor(
                                out=kb[:], in0=kb[:], in1=rl[:], op=ALU.subtract)
                            nc.scalar.activation(
                                kp_blk[:, t4, gsl], kb[:], AF.Exp)
                            nc.vector.tensor_tensor(
                                out=kp_blk[:, t4, gsl], in0=kp_blk[:, t4, gsl],
                                in1=rl[:], op=ALU.add)
                        else:
                            nc.vector.tensor_tensor(
                                out=v_blk[:, t4, gsl], in0=ps[:],
                                in1=bvb[:, gsl], op=ALU.add)

                # ksum (free-major) accumulated across whole P1:
                # ks_ps[ec][0, :] += ones^T @ kp
                for t4 in range(4):
                    for ec in range(2):
                        nc.tensor.matmul(
                            ks_ps[ec][:], ones2_t[:], kp_blk[:, t4, ts(ec, 512)],
                            start=(blk == 0 and t4 == 0),
                            stop=(blk == NBLK - 1 and t4 == 3))

                for dc in range(DCH):
                    dsl = ts(dc, 128)
                    for ec in range(2):
                        esl = ts(ec, 512)
                        pkv = ps_kv.tile([128, 512], F32,
                                         name=f"pkv{blk}_{dc}_{ec}", tag="ps_kv")
                        for t4 in range(4):
                            nc.tensor.matmul(
                                pkv[:], kp_blk[:, t4, dsl],
                                v_blk[:, t4, esl],
                                start=(t4 == 0), stop=(t4 == 3))
                        if blk == 0:
                            nc.vector.tensor_copy(
                                out=kv_acc[:, dc, esl], in_=pkv[:])
                        else:
                            nc.vector.tensor_tensor(
                                out=kv_acc[:, dc, esl], in0=kv_acc[:, dc, esl],
                                in1=pkv[:], op=ALU.add)

            # ksum: psum [2, 1024] free-major -> ksum_acc [128, dc, 2]
            ks_row = ks_p.tile([2, D], F32, name="ks_row")
            for ec in range(2):
                nc.scalar.copy(out=ks_row[:, ts(ec, 512)], in_=ks_ps[ec][:])
            for dc in range(DCH):
                ptk = ps_proj.tile([128, 2], F32, name=f"ptk{dc}", tag="ps_proj")
                nc.tensor.transpose(ptk[:], ks_row[:, ts(dc, 128)],
                                    ident[0:2, 0:2])
                nc.scalar.copy(out=ksum_acc[:, dc, :], in_=ptk[:])

        # ---- pair AllReduce of (kv | ksum) in bf16 -----------------------
        kv_ci = dram.tile([128, DCH, D + 2], BF16, name="kv_ci")
        kv_co = dram.tile([128, DCH, D + 2], BF16, name="kv_co")
        nc.sync.dma_start(out=kv_ci[:][:, :, 0:D], in_=kv_acc[:])
        nc.sync.dma_start(out=kv_ci[:][:, :, D:D + 2], in_=ksum_acc[:])
        nc.gpsimd.collective_compute(
            "AllReduce", ALU.add,
            ins=[kv_ci[:]], outs=[kv_co[:]], replica_groups=GROUPS)
        # result DMAs on idle SWDGE rings (Pool stream is empty here);
        # ksum first and kv in halves so den/num-ec0 unblock ASAP
        nc.gpsimd.dma_start(out=ksum_acc[:], in_=kv_co[:][:, :, D:D + 2])
        nc.gpsimd.dma_start(out=kv_acc[:, :, 0:512],
                            in_=kv_co[:][:, :, 0:512])
        nc.gpsimd.dma_start(out=kv_acc[:, :, 512:1024],
                            in_=kv_co[:][:, :, 512:1024])

        # ---------------- P2: q/num/den/attn/LN1/transpose per tgroup -----
        with ExitStack() as p2:
            c2_p = p2.enter_context(tc.tile_pool(name="c2", bufs=1))
            xg_p = p2.enter_context(tc.tile_pool(name="xg", bufs=2))
            qp_p = p2.enter_context(tc.tile_pool(name="qp", bufs=4))
            xtok_p = p2.enter_context(tc.tile_pool(name="xtok", bufs=4))
            h1_p = p2.enter_context(tc.tile_pool(name="h1", bufs=8))
            x1_p = p2.enter_context(tc.tile_pool(name="x1", bufs=4))
            x1T_p = p2.enter_context(tc.tile_pool(name="x1T", bufs=2))
            tmp2_p = p2.enter_context(tc.tile_pool(name="p2tmp", bufs=4))
            st_p = p2.enter_context(tc.tile_pool(name="p2stat", bufs=4))
            den_p = p2.enter_context(tc.tile_pool(name="denp", bufs=2))
            ps_proj2 = p2.enter_context(
                tc.tile_pool(name="ps_proj2", bufs=3, space="PSUM"))
            ps_den = p2.enter_context(
                tc.tile_pool(name="ps_den", bufs=1, space="PSUM"))
            ps_num = p2.enter_context(
                tc.tile_pool(name="ps_num", bufs=2, space="PSUM"))
            ps_tr = p2.enter_context(
                tc.tile_pool(name="ps_tr", bufs=2, space="PSUM"))

            # q weights split into 4 column chunks (prefetch-friendly);
            # allocated last so the pool can be popped once projections done
            wq_sc = ExitStack()
            if NTG > 4:
                p2.enter_context(wq_sc)
            wq_p = wq_sc.enter_context(tc.tile_pool(name="wqp", bufs=4))
            wq_c = []
            for j in range(4):
                t = wq_p.tile([128, DCH, 256], F32R, name=f"wq{j}", tag="wqc")
                nc.scalar.dma_start(out=t[:], in_=wq_v[:, :, ts(j, 256)])
                wq_c.append(t)
            g1b = c2_p.tile([128, D], F32, name="g1b")
            nc.sync.dma_start(out=g1b[:], in_=_bcast_row(nc, g1_row))
            b1b = c2_p.tile([128, D], F32, name="b1b")
            nc.sync.dma_start(out=b1b[:], in_=_bcast_row(nc, b1_row))

            def emit_qproj(tg):
                o = tg * 512
                xg = xg_p.tile([128, DCH, 512], F32R, name=f"xg{tg}", tag="xg")
                nc.sync.dma_start(out=xg[:], in_=x_ownT_v[:, :, o:o + 512])
                qp_g = qp_p.tile([128, DCH, 512], BF16, name=f"qpg{tg}", tag="qp")
                for qc in range(DCH):
                    ps = ps_proj2.tile([128, 512], F32, name=f"pq{tg}_{qc}",
                                       tag="ps_proj2")
                    for dc in range(DCH):
                        nc.tensor.matmul(
                            ps[:], wq_c[qc // 2][:, dc, ts(qc % 2, 128)],
                            xg[:, dc, :],
                            start=(dc == 0), stop=(dc == DCH - 1))
                    bql = bq_s[:, qc:qc + 1]
                    rl = tmp2_p.tile([128, 512], F32, name=f"qr{tg}_{qc}",
                                     tag="qr")
                    nc.scalar.activation(rl[:], ps[:], AF.Relu, bias=bql)
                    mm = tmp2_p.tile([128, 512], F32, name=f"qm{tg}_{qc}",
                                     tag="qm")
                    nc.vector.scalar_tensor_tensor(
                        out=mm[:], in0=ps[:], scalar=bql, in1=rl[:],
                        op0=ALU.add, op1=ALU.subtract)
                    nc.scalar.activation(qp_g[:, qc, :], mm[:], AF.Exp)
                    nc.vector.tensor_tensor(
                        out=qp_g[:, qc, :], in0=qp_g[:, qc, :], in1=rl[:],
                        op=ALU.add)
                return qp_g

            qp_queue = [emit_qproj(t) for t in range(min(4, NTG))]
            if NTG <= 4:
                wq_sc.close()  # free q-weight SBUF once all projections queued

            for pr in range(0, NTG, 2):
                tgs = [t for t in (pr, pr + 1) if t < NTG]
                den_cs = {}
                for tg in tgs:
                    qp_g = qp_queue[tg]
                    # den for whole tgroup: [2, 512] = ksum^T @ qp
                    pdn = ps_den.tile([2, 512], F32, name=f"pdn{tg}",
                                      tag="ps_den")
                    for dc in range(DCH):
                        nc.tensor.matmul(
                            pdn[:], ksum_acc[:, dc, :], qp_g[:, dc, :],
                            start=(dc == 0), stop=(dc == DCH - 1))
                    den_sb = den_p.tile([2, 512], F32, name=f"dnr{tg}",
                                        tag="dnr")
                    nc.vector.tensor_scalar_add(
                        out=den_sb[:], in0=pdn[:], scalar1=ATTN_EPS)
                    nc.vector.reciprocal(out=den_sb[:], in_=den_sb[:])
                    den_c = den_p.tile([128, 4, 2], F32, name=f"dnc{tg}",
                                       tag="dnc")
                    for t4 in range(4):
                        ptd = ps_tr.tile([128, 2], F32, name=f"ptd{tg}_{t4}",
                                         tag="ps_tr")
                        nc.tensor.transpose(ptd[:], den_sb[:, ts(t4, 128)],
                                            ident[0:2, 0:2])
                        nc.scalar.copy(out=den_c[:, t4, :], in_=ptd[:])
                    den_cs[tg] = den_c

                # num in e-chunk phases: ec0 for both tgroups hides the
                # second AllReduce chunk; ec1 follows.
                h1s = {}
                for ec in range(2):
                    esl = ts(ec, 512)
                    for tg in tgs:
                        qp_g = qp_queue[tg]
                        o = tg * 512
                        for t4 in range(4):
                            tok = o + t4 * 128
                            if ec == 0:
                                h1s[(tg, t4)] = h1_p.tile(
                                    [128, D], F32, name=f"h1_{tg}_{t4}",
                                    tag="h1")
                            xth = xtok_p.tile([128, 512], F32,
                                              name=f"xt{tg}_{t4}_{ec}",
                                              tag="xtok")
                            nc.sync.dma_start(
                                out=xth[:],
                                in_=x_own[tok:tok + 128, esl])
                            pn = ps_num.tile([128, 512], F32,
                                             name=f"pn{tg}_{t4}_{ec}",
                                             tag="ps_num")
                            for dc in range(DCH):
                                nc.tensor.matmul(
                                    pn[:], qp_g[:, dc, ts(t4, 128)],
                                    kv_acc[:, dc, esl],
                                    start=(dc == 0), stop=(dc == DCH - 1))
                            nc.vector.scalar_tensor_tensor(
                                out=h1s[(tg, t4)][:, esl], in0=pn[:],
                                scalar=den_cs[tg][:, t4, 0:1],
                                in1=xth[:], op0=ALU.mult, op1=ALU.add)

                # LN1 + transpose
                for tg in tgs:
                    o = tg * 512
                    if tg + 4 < NTG:
                        qp_queue.append(emit_qproj(tg + 4))
                    x1ns = []
                    for t4 in range(4):
                        tok = o + t4 * 128
                        h1 = h1s[(tg, t4)]
                        stats = st_p.tile([128, 2, 6], F32,
                                          name=f"s1_{tg}_{t4}", tag="st1")
                        nc.vector.bn_stats(out=stats[:, 0, :], in_=h1[:, 0:512])
                        nc.vector.bn_stats(out=stats[:, 1, :],
                                           in_=h1[:, 512:1024])
                        mv = st_p.tile([128, 2], F32, name=f"mv1_{tg}_{t4}",
                                       tag="mv1")
                        nc.vector.bn_aggr(out=mv[:], in_=stats[:])
                        rstd = st_p.tile([128, 1], F32, name=f"rs1_{tg}_{t4}",
                                         tag="rstd1")
                        nc.scalar.activation(rstd[:], mv[:, 1:2], AF.Sqrt,
                                             bias=epsb[:])
                        nc.vector.reciprocal(out=rstd[:], in_=rstd[:])
                        x1n = x1_p.tile([128, D], F32, name=f"x1_{tg}_{t4}",
                                        tag="x1")
                        nc.vector.tensor_scalar(
                            out=x1n[:], in0=h1[:], scalar1=mv[:, 0:1],
                            scalar2=rstd[:], op0=ALU.subtract, op1=ALU.mult)
                        nc.vector.tensor_tensor(
                            out=x1n[:], in0=x1n[:], in1=g1b[:], op=ALU.mult)
                        nc.vector.tensor_tensor(
                            out=x1n[:], in0=x1n[:], in1=b1b[:], op=ALU.add)
                        nc.sync.dma_start(out=x1_dram[tok:tok + 128, :],
                                          in_=x1n[:])
                        x1ns.append(x1n)

                    for t4 in range(4):
                        tok = o + t4 * 128
                        x1n = x1ns[t4]
                        x1T_t = x1T_p.tile([128, DCH, 128], F32R,
                                           name=f"x1T{tg}_{t4}", tag="x1T")
                        for dc in range(DCH):
                            pt = ps_tr.tile([128, 128], F32,
                                            name=f"pt{tg}_{t4}_{dc}",
                                            tag="ps_tr")
                            nc.tensor.transpose(pt[:], x1n[:, ts(dc, 128)],
                                                ident[:])
                            nc.scalar.copy(out=x1T_t[:, dc, :], in_=pt[:])
                        nc.sync.dma_start(
                            out=x1T_dram_v[:, :, tok:tok + 128], in_=x1T_t[:])

        p12.close()  # release kv/ksum accumulators before P3

        # ---------------- P3: FFN + LN2 in 1024-token passes --------------
        NPASS = (T_OWN + 1023) // 1024
        with ExitStack() as p3:
            c3_p = p3.enter_context(tc.tile_pool(name="c3", bufs=1))
            bdb = c3_p.tile([128, D], F32, name="bdb")
            nc.sync.dma_start(out=bdb[:], in_=_bcast_row(nc, bd_row))
            g2b = c3_p.tile([128, D], F32, name="g2b")
            nc.sync.dma_start(out=g2b[:], in_=_bcast_row(nc, g2_row))
            b2b = c3_p.tile([128, D], F32, name="b2b")
            nc.sync.dma_start(out=b2b[:], in_=_bcast_row(nc, b2_row))
            ffn_p = p3.enter_context(tc.tile_pool(name="ffn", bufs=1))

            for ps_i in range(NPASS):
                p0 = ps_i * 1024
                ptok = min(1024, T_OWN - p0)
                ntgh = ptok // 512
                ffn_t = ffn_p.tile([128, HCH, ptok], F32R, name=f"ffn{ps_i}",
                                   tag="ffn")
                wsc = ExitStack()
                wd_p = wsc.enter_context(tc.tile_pool(name="wdp", bufs=6))

                with ExitStack() as gsc:
                    x1T_q = gsc.enter_context(tc.tile_pool(name="x1Tq", bufs=2))
                    wgu_p = gsc.enter_context(tc.tile_pool(name="wgu", bufs=2))
                    sg_p = gsc.enter_context(tc.tile_pool(name="sg", bufs=2))
                    ps_g = gsc.enter_context(
                        tc.tile_pool(name="ps_g", bufs=2, space="PSUM"))
                    ps_u = gsc.enter_context(
                        tc.tile_pool(name="ps_u", bufs=2, space="PSUM"))
                    x1Ts = []
                    for tgh in range(ntgh):
                        t = x1T_q.tile([128, DCH, 512], F32R,
                                       name=f"x1Tq{ps_i}_{tgh}", tag="x1Tq")
                        o = p0 + tgh * 512
                        nc.gpsimd.dma_start(out=t[:],
                                            in_=x1T_dram_v[:, :, o:o + 512])
                        x1Ts.append(t)
                    for hd in range(HCH // 2):
                        csl = ts(hd, 256)
                        wg_d = wgu_p.tile([128, DCH, 256], F32R,
                                          name=f"wgd{ps_i}_{hd}", tag="wgd")
                        nc.gpsimd.dma_start(out=wg_d[:], in_=wg_v[:, :, csl])
                        wu_d = wgu_p.tile([128, DCH, 256], F32R,
                                          name=f"wud{ps_i}_{hd}", tag="wud")
                        nc.gpsimd.dma_start(out=wu_d[:], in_=wu_v[:, :, csl])
                        for hl in range(2):
                            hc = hd * 2 + hl
                            for tgh in range(ntgh):
                                x1T_t = x1Ts[tgh]
                                fsl = ts(tgh, 512)
                                psg = ps_g.tile([128, 512], F32,
                                                name=f"pg{ps_i}_{hc}_{tgh}",
                                                tag="ps_g")
                                for dc in range(DCH):
                                    nc.tensor.matmul(
                                        psg[:], wg_d[:, dc, ts(hl, 128)],
                                        x1T_t[:, dc, :],
                                        start=(dc == 0), stop=(dc == DCH - 1))
                                psu = ps_u.tile([128, 512], F32,
                                                name=f"pu{ps_i}_{hc}_{tgh}",
                                                tag="ps_u")
                                for dc in range(DCH):
                                    nc.tensor.matmul(
                                        psu[:], wu_d[:, dc, ts(hl, 128)],
                                        x1T_t[:, dc, :],
                                        start=(dc == 0), stop=(dc == DCH - 1))
                                bgl = bg_s[:, hc:hc + 1]
                                sig = sg_p.tile([128, 512], F32,
                                                name=f"sig{ps_i}_{hc}_{tgh}",
                                                tag="sig")
                                nc.scalar.activation(sig[:], psg[:], AF.Sigmoid,
                                                     bias=bgl)
                                gate = sg_p.tile([128, 512], F32,
                                                 name=f"gt{ps_i}_{hc}_{tgh}",
                                                 tag="gate")
                                nc.vector.tensor_scalar_add(
                                    out=gate[:], in0=psg[:], scalar1=bgl)
                                nc.vector.tensor_tensor(
                                    out=gate[:], in0=gate[:], in1=sig[:],
                                    op=ALU.mult)
                                nc.vector.scalar_tensor_tensor(
                                    out=ffn_t[:, hc, fsl], in0=psu[:],
                                    scalar=bu_s[:, hc:hc + 1], in1=gate[:],
                                    op0=ALU.add, op1=ALU.mult)

                with wsc, ExitStack() as dsc:
                    x1r_p = dsc.enter_context(tc.tile_pool(name="x1r", bufs=8))
                    h2_p = dsc.enter_context(tc.tile_pool(name="h2", bufs=8))
                    st3_p = dsc.enter_context(tc.tile_pool(name="p3stat", bufs=4))
                    out_p = dsc.enter_context(tc.tile_pool(name="outp", bufs=2))
                    ps_dn = dsc.enter_context(
                        tc.tile_pool(name="ps_dn", bufs=8, space="PSUM"))

                    nt8 = ptok // 128
                    x1r = []
                    for t8 in range(nt8):
                        tok = p0 + t8 * 128
                        xr = x1r_p.tile([128, D], F32, name=f"x1r{ps_i}_{t8}",
                                        tag="x1r")
                        nc.gpsimd.dma_start(out=xr[:],
                                            in_=x1_dram[tok:tok + 128, :])
                        nc.vector.tensor_tensor(
                            out=xr[:], in0=xr[:], in1=bdb[:], op=ALU.add)
                        x1r.append(xr)

                    h2 = [h2_p.tile([128, D], F32, name=f"h2_{ps_i}_{t8}",
                                    tag="h2") for t8 in range(nt8)]
                    if ps_i == NPASS - 1 and nt8 > 4:
                        t8_groups = [list(range(0, 4)), list(range(4, nt8))]
                    else:
                        t8_groups = [list(range(nt8))]
                    for t8g in t8_groups:
                        for dg in range(2):
                            dsl = ts(dg, 512)
                            psd = {t8: ps_dn.tile([128, 512], F32,
                                                  name=f"pd{ps_i}_{dg}_{t8}",
                                                  tag="ps_dn") for t8 in t8g}
                            for hc in range(HCH):
                                wd_t = wd_p.tile(
                                    [128, 512], F32R,
                                    name=f"wdt{ps_i}_{t8g[0]}_{dg}_{hc}",
                                    tag="wdt")
                                nc.sync.dma_start(
                                    out=wd_t[:], in_=wd[ts(hc, 128), dsl])
                                for t8 in t8g:
                                    nc.tensor.matmul(
                                        psd[t8][:], ffn_t[:, hc, ts(t8, 128)],
                                        wd_t[:],
                                        start=(hc == 0), stop=(hc == HCH - 1))
                            for t8 in t8g:
                                nc.vector.tensor_tensor(
                                    out=h2[t8][:, dsl], in0=psd[t8][:],
                                    in1=x1r[t8][:, dsl], op=ALU.add)

                    # LN2 + store
                    for t8 in range(nt8):
                        tok = p0 + t8 * 128
                        stats = st3_p.tile([128, 2, 6], F32,
                                           name=f"s2_{ps_i}_{t8}", tag="st2")
                        nc.vector.bn_stats(out=stats[:, 0, :],
                                           in_=h2[t8][:, 0:512])
                        nc.vector.bn_stats(out=stats[:, 1, :],
                                           in_=h2[t8][:, 512:1024])
                        mv = st3_p.tile([128, 2], F32, name=f"mv2_{ps_i}_{t8}",
                                        tag="mv2")
                        nc.vector.bn_aggr(out=mv[:], in_=stats[:])
                        rstd = st3_p.tile([128, 1], F32,
                                          name=f"rs2_{ps_i}_{t8}", tag="rstd2")
                        nc.scalar.activation(rstd[:], mv[:, 1:2], AF.Sqrt,
                                             bias=epsb[:])
                        nc.vector.reciprocal(out=rstd[:], in_=rstd[:])
                        o_t = out_p.tile([128, D], F32, name=f"o{ps_i}_{t8}",
                                         tag="ot")
                        nc.vector.tensor_scalar(
                            out=o_t[:], in0=h2[t8][:], scalar1=mv[:, 0:1],
                            scalar2=rstd[:], op0=ALU.subtract, op1=ALU.mult)
                        nc.vector.tensor_tensor(
                            out=o_t[:], in0=o_t[:], in1=g2b[:], op=ALU.mult)
                        nc.vector.tensor_tensor(
                            out=o_t[:], in0=o_t[:], in1=b2b[:], op=ALU.add)
                        nc.sync.dma_start(out=out[tok:tok + 128, :], in_=o_t[:])

    nc.compile()
    return nc, input_names


# ---------------------------------------------------------------------------
# Host-side wrapper
# ---------------------------------------------------------------------------

B, S, D_MODEL, D_FF = 4, 4096, 1024, 4096
FFN_H = int(2 * D_FF / 3)  # 2730

_cache = {}
LAST_RESULTS = None


def _get_program(T_OWN=2048, T_FULL=4096):
    key = (T_OWN, T_FULL)
    if key not in _cache:
        _cache[key] = build_program(T_OWN, T_FULL)
    return _cache[key]


def _prep_shared(Wqkv, bqkv, Wg, bg, Wu, bu, Wd, bd, g1, b1, g2, b2):
    f = np.float32
    Wqkv = np.asarray(Wqkv, f)
    sh = {}
    sh["wq"] = np.ascontiguousarray(Wqkv[:, 0:1024])
    sh["wk"] = np.ascontiguousarray(Wqkv[:, 1024:2048])
    sh["wv"] = np.ascontiguousarray(Wqkv[:, 2048:3072])
    bqkv = np.asarray(bqkv, f)
    sh["bq_pre"] = np.ascontiguousarray(bqkv[0:1024].reshape(8, 128).T)
    sh["bk_row"] = np.ascontiguousarray(bqkv[1024:2048].reshape(1, 1024))
    sh["bv_row"] = np.ascontiguousarray(bqkv[2048:3072].reshape(1, 1024))
    wg_p = np.zeros((1024, H_PAD), f)
    wg_p[:, :FFN_H] = np.asarray(Wg, f)
    sh["wg"] = wg_p
    wu_p = np.zeros((1024, H_PAD), f)
    wu_p[:, :FFN_H] = np.asarray(Wu, f)
    sh["wu"] = wu_p
    bg_p = np.zeros((H_PAD,), f)
    bg_p[:FFN_H] = np.asarray(bg, f)
    sh["bg_pre"] = np.ascontiguousarray(bg_p.reshape(HCH, 128).T)
    bu_p = np.zeros((H_PAD,), f)
    bu_p[:FFN_H] = np.asarray(bu, f)
    sh["bu_pre"] = np.ascontiguousarray(bu_p.reshape(HCH, 128).T)
    wd_p = np.zeros((H_PAD, 1024), f)
    wd_p[:FFN_H, :] = np.asarray(Wd, f)
    sh["wd"] = wd_p
    sh["bd_row"] = np.asarray(bd, f).reshape(1, 1024)
    sh["g1_row"] = np.asarray(g1, f).reshape(1, 1024)
    sh["b1_row"] = np.asarray(b1, f).reshape(1, 1024)
    sh["g2_row"] = np.asarray(g2, f).reshape(1, 1024)
    sh["b2_row"] = np.asarray(b2, f).reshape(1, 1024)
    o2 = np.zeros((128, 2), f); o2[:, 0] = 1.0; sh["ones2"] = o2
    return sh


def make_in_maps(x, Wqkv, bqkv, Wg, bg, Wu, bu, Wd, bd, g1, b1, g2, b2):
    x = np.asarray(x, np.float32)
    sh = _prep_shared(Wqkv, bqkv, Wg, bg, Wu, bu, Wd, bd, g1, b1, g2, b2)
    in_maps = []
    for c in range(8):
        b, h = c // 2, c % 2
        m = dict(sh)
        m["x_ownT"] = np.ascontiguousarray(x[b, h * 2048:(h + 1) * 2048].T)
        m["x_own"] = np.ascontiguousarray(x[b, h * 2048:(h + 1) * 2048])
        in_maps.append(m)
    return in_maps


def kernel(x, Wqkv, bqkv, Wg, bg, Wu, bu, Wd, bd, g1, b1, g2, b2):
    global LAST_RESULTS
    from concourse import bass_utils

    nc, _names = _get_program()
    in_maps = make_in_maps(x, Wqkv, bqkv, Wg, bg, Wu, bu, Wd, bd,
                           g1, b1, g2, b2)
    res = bass_utils.run_bass_kernel_spmd(nc, in_maps, core_ids=list(range(8)))
    LAST_RESULTS = res
    out = np.empty((B, S, D_MODEL), np.float32)
    for c in range(8):
        b, h = c // 2, c % 2
        out[b, h * 2048:(h + 1) * 2048] = res.results[c]["out"]
    return out



# revision 15
# speedup vs baseline: 1.0722x; 1.0722x over previous
"""Trainium2 Bass kernel for nn_Block_54219667145535 (linear-attention block).

v2: fully-transposed (feature-major) bf16 pipeline.

Sharding: 8 cores, 2 per batch (B=4). Each core computes k/v projections +
partial [D,D] kv state over its own 2048 tokens, pair-AllReduces the packed
(kv|ksum) buffer (single contiguous 16.4KB/partition descriptor), and hides
the collective under the q projection. Everything downstream (num, LN1,
SwiGLU FFN, LN2, output) stays in [d-partition, token] orientation, so no PE
transposes and no DRAM round-trip for x1. The host pre-transposes x and all
weights into the layouts the kernel wants and re-transposes the output.
"""

import os
import sys
from contextlib import ExitStack

import numpy as np


def _ensure_paths():
    for p in ("/opt/trn_rl_repo", "/root/.axon_site/_ro/trn_rl_repo"):
        if os.path.isdir(p) and p not in sys.path:
            sys.path.insert(0, p)
    try:
        import concourse.bass  # noqa: F401
    except ImportError as e:  # pragma: no cover
        raise ImportError(f"concourse not importable: {e}")


_ensure_paths()

import ml_dtypes  # noqa: E402

import concourse.bacc as bacc  # noqa: E402
import concourse.tile as tile  # noqa: E402
from concourse import mybir  # noqa: E402
from concourse.bass import ts  # noqa: E402

F32 = mybir.dt.float32
BF16 = mybir.dt.bfloat16
AF = mybir.ActivationFunctionType
ALU = mybir.AluOpType

D = 1024
DCH = 8          # d chunks of 128
H_PAD = 2816
HCH = 22         # h chunks of 128
HBLK = 11        # h blocks of 256 (for weight streaming)
LN_EPS = 1e-5
ATTN_EPS = 1e-6
KV_W = DCH * D   # 8192 bf16 kv values per partition
PK_W = KV_W + DCH  # + 8 ksum values


def build_program(T_OWN=2048, n_cores=8):
    """Per-core Bass/Tile program. Pair (2b, 2b+1) handles batch b."""
    NBLK = T_OWN // 512   # P1 token blocks
    NTG = T_OWN // 512    # P2/P3 token groups
    GROUPS = [[c, c + 1] for c in range(0, n_cores, 2)]

    nc = bacc.Bacc(
        "TRN2",
        target_bir_lowering=False,
        debug=False,
        enable_asserts=False,
        num_devices=8,
        num_swdge_queues=4,
    )

    # ---- I/O (host supplies pre-transposed / pre-chunked layouts) ----
    # xh[p, c, t] = x[t, c*128+p]
    xh = nc.dram_tensor("xh", [128, DCH, T_OWN], BF16, kind="ExternalInput").ap()
    wk = nc.dram_tensor("wk", [128, DCH, D], BF16, kind="ExternalInput").ap()
    wv = nc.dram_tensor("wv", [128, DCH, D], BF16, kind="ExternalInput").ap()
    wq = nc.dram_tensor("wq", [128, DCH, D], BF16, kind="ExternalInput").ap()
    bk_row = nc.dram_tensor("bk_row", [1, D], BF16, kind="ExternalInput").ap()
    bv_row = nc.dram_tensor("bv_row", [1, D], BF16, kind="ExternalInput").ap()
    bq_col = nc.dram_tensor("bq_col", [128, DCH], F32, kind="ExternalInput").ap()
    # wg/wu: [p, hb, c, hsub]: lhsT chunk for (hb, dc, hl) = [:, hb, dc, 256]
    wg = nc.dram_tensor("wg", [128, HBLK, DCH, 256], BF16,
                        kind="ExternalInput").ap()
    wu = nc.dram_tensor("wu", [128, HBLK, DCH, 256], BF16,
                        kind="ExternalInput").ap()
    bg_col = nc.dram_tensor("bg_col", [128, HCH], F32, kind="ExternalInput").ap()
    bu_col = nc.dram_tensor("bu_col", [128, HCH], F32, kind="ExternalInput").ap()
    # wd[p, hc, d] = Wd[hc*128+p, d]
    wd = nc.dram_tensor("wd", [128, HCH, D], BF16, kind="ExternalInput").ap()
    bd_col = nc.dram_tensor("bd_col", [128, DCH], F32, kind="ExternalInput").ap()
    g1_col = nc.dram_tensor("g1_col", [128, DCH], F32, kind="ExternalInput").ap()
    b1_col = nc.dram_tensor("b1_col", [128, DCH], F32, kind="ExternalInput").ap()
    g2_col = nc.dram_tensor("g2_col", [128, DCH], F32, kind="ExternalInput").ap()
    b2_col = nc.dram_tensor("b2_col", [128, DCH], F32, kind="ExternalInput").ap()
    # outT[(c p), t] = out[t, c*128+p]
    out = nc.dram_tensor("out", [D, T_OWN], BF16, kind="ExternalOutput").ap()
    out_v = out.rearrange("(c p) t -> p c t", p=128)

    input_names = [
        "xh", "wk", "wv", "wq", "bk_row", "bv_row", "bq_col", "wg", "wu",
        "bg_col", "bu_col", "wd", "bd_col", "g1_col", "b1_col", "g2_col",
        "b2_col",
    ]

    def bcast_row(row_ap, parts=128):
        import concourse.bass as bass
        return bass.AP(
            tensor=row_ap.tensor,
            offset=row_ap.offset,
            ap=[[0, parts]] + [list(d) for d in row_ap.ap[1:]],
        )

    with tile.TileContext(nc) as tc, ExitStack() as top:
        dram = top.enter_context(tc.tile_pool(name="dram", bufs=1, space="DRAM"))
        kv_pack = dram.tile([128, PK_W], BF16, name="kv_pack")
        kv_out = dram.tile([128, PK_W], BF16, name="kv_out")

        consts = top.enter_context(tc.tile_pool(name="consts", bufs=1))
        ones_col = consts.tile([128, 1], BF16, name="ones_col")
        nc.vector.memset(ones_col[:], 1.0)
        ones_row = consts.tile([1, 128], BF16, name="ones_row")
        nc.vector.memset(ones_row[:], 1.0)
        one1 = consts.tile([1, 1], F32, name="one1")
        nc.vector.memset(one1[:], 1.0)
        epsr = consts.tile([1, 1], F32, name="epsr")
        nc.vector.memset(epsr[:], LN_EPS)
        bq_s = consts.tile([128, DCH], F32, name="bq_s")
        nc.sync.dma_start(out=bq_s[:], in_=bq_col)
        nbq_s = consts.tile([128, DCH], F32, name="nbq_s")
        nc.vector.tensor_scalar_mul(out=nbq_s[:], in0=bq_s[:], scalar1=-1.0)
        bg_s = consts.tile([128, HCH], F32, name="bg_s")
        nc.sync.dma_start(out=bg_s[:], in_=bg_col)
        bu_s = consts.tile([128, HCH], F32, name="bu_s")
        nc.sync.dma_start(out=bu_s[:], in_=bu_col)
        bd_s = consts.tile([128, DCH], F32, name="bd_s")
        nc.sync.dma_start(out=bd_s[:], in_=bd_col)
        g1_s = consts.tile([128, DCH], F32, name="g1_s")
        nc.sync.dma_start(out=g1_s[:], in_=g1_col)
        b1_s = consts.tile([128, DCH], F32, name="b1_s")
        nc.sync.dma_start(out=b1_s[:], in_=b1_col)
        g2_s = consts.tile([128, DCH], F32, name="g2_s")
        nc.sync.dma_start(out=g2_s[:], in_=g2_col)
        b2_s = consts.tile([128, DCH], F32, name="b2_s")
        nc.sync.dma_start(out=b2_s[:], in_=b2_col)

        # x1 output of LN1 — outlives P2, consumed by P3 (alloc first: LIFO)
        x1res = top.enter_context(tc.tile_pool(name="x1res", bufs=1))
        x1T = x1res.tile([128, DCH, T_OWN], BF16, name="x1T")

        # x resident through P1+P2 (32KB/partition), freed before P3
        xres = ExitStack()
        xres_p = xres.enter_context(tc.tile_pool(name="xres", bufs=1))
        x_sb = xres_p.tile([128, DCH, T_OWN], BF16, name="x_sb")
        for blk in range(NBLK):
            nc.sync.dma_start(out=x_sb[:, :, ts(blk, 512)],
                              in_=xh[:, :, ts(blk, 512)])

        # kv+ksum accumulator, also the collective staging target
        p12 = ExitStack()
        accs = p12.enter_context(tc.tile_pool(name="accs", bufs=1))
        kv_sb = accs.tile([128, PK_W], BF16, name="kv_sb")
        kv_v = kv_sb[:][:, 0:KV_W].rearrange("p (c e) -> p c e", c=DCH)
        ks_v = kv_sb[:][:, KV_W:PK_W].rearrange("p (c e) -> p c e", c=DCH)

        # ---------------- P1: k/v proj + kv/ksum over own tokens ----------
        with ExitStack() as p1:
            wkv_p = p1.enter_context(tc.tile_pool(name="wkv", bufs=1))
            c1_p = p1.enter_context(tc.tile_pool(name="c1", bufs=1))
            kpv_p = p1.enter_context(tc.tile_pool(name="kpv", bufs=2))
            tmp_p = p1.enter_context(tc.tile_pool(name="p1tmp", bufs=3))
            ksr_p = p1.enter_context(tc.tile_pool(name="ksr", bufs=1))
            ps_proj = p1.enter_context(
                tc.tile_pool(name="ps_proj", bufs=3, space="PSUM"))
            ps_kv = p1.enter_context(
                tc.tile_pool(name="ps_kv", bufs=3, space="PSUM"))
            ps_ks = p1.enter_context(
                tc.tile_pool(name="ps_ks", bufs=2, space="PSUM"))

            wk_s = wkv_p.tile([128, DCH, D], BF16, name="wk_s")
            wv_s = wkv_p.tile([128, DCH, D], BF16, name="wv_s")
            for half in range(2):
                nc.scalar.dma_start(out=wk_s[:, :, ts(half, 512)],
                                    in_=wk[:, :, ts(half, 512)])
                nc.scalar.dma_start(out=wv_s[:, :, ts(half, 512)],
                                    in_=wv[:, :, ts(half, 512)])
            bkb = c1_p.tile([128, D], BF16, name="bkb")
            nc.sync.dma_start(out=bkb[:], in_=bcast_row(bk_row))
            bvb = c1_p.tile([128, D], BF16, name="bvb")
            nc.sync.dma_start(out=bvb[:], in_=bcast_row(bv_row))

            ks_ps = [ps_ks.tile([1, 512], F32, name=f"ksps{e}", tag="ps_ks")
                     for e in range(2)]

            for blk in range(NBLK):
                kp_blk = kpv_p.tile([128, 4, D], BF16, name=f"kp{blk}",
                                    tag="kp")
                v_blk = kpv_p.tile([128, 4, D], BF16, name=f"v{blk}", tag="v")

                for t4 in range(4):
                    tok0 = blk * 512 + t4 * 128
                    for which in range(2):  # 0 = k, 1 = v
                        w_s = wk_s if which == 0 else wv_s
                        for half in range(2):
                            gsl = ts(half, 512)
                            ps = ps_proj.tile(
                                [128, 512], F32,
                                name=f"pp{blk}_{t4}_{which}_{half}",
                                tag="ps_proj")
                            for dc in range(DCH):
                                nc.tensor.matmul(
                                    ps[:], x_sb[:, dc, tok0:tok0 + 128],
                                    w_s[:, dc, gsl],
                                    start=(dc == 0), stop=(dc == DCH - 1))
                            if which == 0:
                                # kp = elu(k+bk)+1 = exp(-r) + kb + r,
                                # r = relu(-kb), kb = k + bk
                                kb = tmp_p.tile([128, 512], F32,
                                                name=f"kb{blk}_{t4}_{half}",
                                                tag="kb")
                                nc.vector.tensor_tensor(
                                    out=kb[:], in0=ps[:], in1=bkb[:, gsl],
                                    op=ALU.add)
                                r = tmp_p.tile([128, 512], F32,
                                               name=f"r{blk}_{t4}_{half}",
                                               tag="r")
                                nc.scalar.activation(r[:], kb[:], AF.Relu,
                                                     scale=-1.0)
                                e = tmp_p.tile([128, 512], F32,
                                               name=f"e{blk}_{t4}_{half}",
                                               tag="e")
                                nc.scalar.activation(e[:], r[:], AF.Exp,
                                                     scale=-1.0)
                                nc.vector.tensor_tensor(
                                    out=kb[:], in0=kb[:], in1=r[:], op=ALU.add)
                                nc.vector.tensor_tensor(
                                    out=kp_blk[:, t4, gsl], in0=kb[:],
                                    in1=e[:], op=ALU.add)
                            else:
                                nc.vector.tensor_tensor(
                                    out=v_blk[:, t4, gsl], in0=ps[:],
                                    in1=bvb[:, gsl], op=ALU.add)

                    # ksum += ones^T @ kp for this t4 (both e halves)
                    for ec in range(2):
                        nc.tensor.matmul(
                            ks_ps[ec][:], ones_col[:],
                            kp_blk[:, t4, ts(ec, 512)],
                            start=(blk == 0 and t4 == 0),
                            stop=(blk == NBLK - 1 and t4 == 3))

                # kv[dc, e-half] += kp_blk^T @ v_blk
                for dc in range(DCH):
                    dsl = ts(dc, 128)
                    for ec in range(2):
                        esl = ts(ec, 512)
                        pkv = ps_kv.tile([128, 512], F32,
                                         name=f"pkv{blk}_{dc}_{ec}",
                                         tag="ps_kv")
                        for t4 in range(4):
                            nc.tensor.matmul(
                                pkv[:], kp_blk[:, t4, dsl],
                                v_blk[:, t4, esl],
                                start=(t4 == 0), stop=(t4 == 3))
                        if blk == 0:
                            nc.vector.tensor_copy(
                                out=kv_v[:, dc, esl], in_=pkv[:])
                        else:
                            nc.vector.tensor_tensor(
                                out=kv_v[:, dc, esl], in0=kv_v[:, dc, esl],
                                in1=pkv[:], op=ALU.add)

            # ksum rows [1,1024] -> per-partition cols kv_sb[:, KV_W:]
            ks_row = ksr_p.tile([1, D], F32, name="ks_row")
            for ec in range(2):
                nc.scalar.copy(out=ks_row[:, ts(ec, 512)], in_=ks_ps[ec][:])
            for dc in range(DCH):
                ptk = ps_proj.tile([128, 1], F32, name=f"ptk{dc}",
                                   tag="ps_proj")
                nc.tensor.transpose(ptk[:], ks_row[:, ts(dc, 128)],
                                    one1[:])
                nc.scalar.copy(out=ks_v[:, dc, :], in_=ptk[:])

        # ---- pair AllReduce of packed (kv | ksum), single descriptor ----
        nc.sync.dma_start(out=kv_pack[:], in_=kv_sb[:])
        nc.gpsimd.collective_compute(
            "AllReduce", ALU.add,
            ins=[kv_pack[:]], outs=[kv_out[:]], replica_groups=GROUPS)
        nc.gpsimd.dma_start(out=kv_sb[:], in_=kv_out[:])

        # ---------------- P2: q proj (hides collective), then attn + LN1 --
        with ExitStack() as p2:
            qp_p = p2.enter_context(tc.tile_pool(name="qp", bufs=1))
            qpT = qp_p.tile([128, DCH, T_OWN], BF16, name="qpT")

            with ExitStack() as qsc:
                wq_p = qsc.enter_context(tc.tile_pool(name="wqp", bufs=1))
                qtmp_p = qsc.enter_context(tc.tile_pool(name="qtmp", bufs=3))
                ps_q = qsc.enter_context(
                    tc.tile_pool(name="ps_q", bufs=3, space="PSUM"))
                wq_s = wq_p.tile([128, DCH, D], BF16, name="wq_s")
                for half in range(2):
                    nc.scalar.dma_start(out=wq_s[:, :, ts(half, 512)],
                                        in_=wq[:, :, ts(half, 512)])
                for tg in range(NTG):
                    tsl = ts(tg, 512)
                    for qc in range(DCH):
                        ps = ps_q.tile([128, 512], F32, name=f"pq{tg}_{qc}",
                                       tag="ps_q")
                        for dc in range(DCH):
                            nc.tensor.matmul(
                                ps[:], wq_s[:, dc, ts(qc, 128)],
                                x_sb[:, dc, tsl],
                                start=(dc == 0), stop=(dc == DCH - 1))
                        # qp = elu(q+bq)+1; q+bq has per-partition bias
                        r = qtmp_p.tile([128, 512], F32, name=f"qr{tg}_{qc}",
                                        tag="qr")
                        nc.scalar.activation(r[:], ps[:], AF.Relu,
                                             scale=-1.0,
                                             bias=nbq_s[:, qc:qc + 1])
                        e = qtmp_p.tile([128, 512], F32, name=f"qe{tg}_{qc}",
                                        tag="qe")
                        nc.scalar.activation(e[:], r[:], AF.Exp, scale=-1.0)
                        t = qtmp_p.tile([128, 512], F32, name=f"qt{tg}_{qc}",
                                        tag="qt")
                        nc.vector.scalar_tensor_tensor(
                            out=t[:], in0=ps[:], scalar=bq_s[:, qc:qc + 1],
                            in1=r[:], op0=ALU.add, op1=ALU.add)
                        nc.vector.tensor_tensor(
                            out=qpT[:, qc, tsl], in0=t[:], in1=e[:],
                            op=ALU.add)

            with ExitStack() as asc:
                h1_p = asc.enter_context(tc.tile_pool(name="h1", bufs=2))
                sq_p = asc.enter_context(tc.tile_pool(name="sq", bufs=2))
                row_p = asc.enter_context(tc.tile_pool(name="rows", bufs=1))
                atmp_p = asc.enter_context(tc.tile_pool(name="atmp", bufs=3))
                dbc_p = asc.enter_context(tc.tile_pool(name="dbc", bufs=2))
                ps_row = asc.enter_context(
                    tc.tile_pool(name="ps_row", bufs=2, space="PSUM"))
                ps_num = asc.enter_context(
                    tc.tile_pool(name="ps_num", bufs=4, space="PSUM"))
                ps_bc = asc.enter_context(
                    tc.tile_pool(name="ps_bc", bufs=2, space="PSUM"))

                for tg in range(NTG):
                    tsl = ts(tg, 512)
                    # den row = ksum^T qp + eps, then reciprocal
                    pdn = ps_row.tile([1, 512], F32, name=f"pdn{tg}",
                                      tag="ps_row")
                    for dc in range(DCH):
                        nc.tensor.matmul(
                            pdn[:], ks_v[:, dc, :], qpT[:, dc, tsl],
                            start=(dc == 0), stop=(dc == DCH - 1))
                    dnr = row_p.tile([1, 512], F32, name=f"dnr{tg}",
                                     tag="dnr")
                    nc.vector.tensor_scalar_add(
                        out=dnr[:], in0=pdn[:], scalar1=ATTN_EPS)
                    nc.vector.reciprocal(out=dnr[:], in_=dnr[:])
                    dnb = row_p.tile([1, 512], BF16, name=f"dnb{tg}",
                                     tag="dnb")
                    nc.vector.tensor_copy(out=dnb[:], in_=dnr[:])
                    # broadcast recip(den) across partitions, stage in SBUF
                    pbc_d = ps_bc.tile([128, 512], F32, name=f"pbcd{tg}",
                                       tag="ps_bc")
                    nc.tensor.matmul(pbc_d[:], ones_row[:], dnb[:],
                                     start=True, stop=True)
                    dbc = dbc_p.tile([128, 512], F32, name=f"dbc{tg}",
                                     tag="dbc")
                    nc.scalar.copy(out=dbc[:], in_=pbc_d[:])

                    h1 = h1_p.tile([128, DCH, 512], BF16, name=f"h1_{tg}",
                                   tag="h1")
                    sq = sq_p.tile([128, DCH, 512], BF16, name=f"sq_{tg}",
                                   tag="sq")
                    for ec in range(DCH):
                        pn = ps_num.tile([128, 512], F32,
                                         name=f"pn{tg}_{ec}", tag="ps_num")
                        for dc in range(DCH):
                            nc.tensor.matmul(
                                pn[:], kv_v[:, dc, ts(ec, 128)],
                                qpT[:, dc, tsl],
                                start=(dc == 0), stop=(dc == DCH - 1))
                        at = atmp_p.tile([128, 512], F32,
                                         name=f"at{tg}_{ec}", tag="at")
                        nc.vector.tensor_tensor(
                            out=at[:], in0=pn[:], in1=dbc[:], op=ALU.mult)
                        nc.vector.tensor_tensor(
                            out=h1[:, ec, :], in0=at[:],
                            in1=x_sb[:, ec, tsl], op=ALU.add)
                        nc.scalar.activation(sq[:, ec, :], h1[:, ec, :],
                                             AF.Square)

                    # LN1 stats: sum & sumsq over d via ones-matmuls
                    psm = ps_row.tile([1, 512], F32, name=f"psm{tg}",
                                      tag="ps_row")
                    for dc in range(DCH):
                        nc.tensor.matmul(psm[:], ones_col[:], h1[:, dc, :],
                                         start=(dc == 0),
                                         stop=(dc == DCH - 1))
                    pss = ps_row.tile([1, 512], F32, name=f"pss{tg}",
                                      tag="ps_row")
                    for dc in range(DCH):
                        nc.tensor.matmul(pss[:], ones_col[:], sq[:, dc, :],
                                         start=(dc == 0),
                                         stop=(dc == DCH - 1))
                    mu = row_p.tile([1, 512], F32, name=f"mu{tg}", tag="mu")
                    nc.vector.tensor_scalar_mul(out=mu[:], in0=psm[:],
                                                scalar1=1.0 / D)
                    ex2 = row_p.tile([1, 512], F32, name=f"ex2{tg}",
                                     tag="ex2")
                    nc.vector.tensor_scalar_mul(out=ex2[:], in0=pss[:],
                                                scalar1=1.0 / D)
                    var = row_p.tile([1, 512], F32, name=f"var{tg}",
                                     tag="var")
                    nc.vector.tensor_tensor(out=var[:], in0=mu[:], in1=mu[:],
                                            op=ALU.mult)
                    nc.vector.tensor_tensor(out=var[:], in0=ex2[:],
                                            in1=var[:], op=ALU.subtract)
                    rstd = row_p.tile([1, 512], F32, name=f"rstd{tg}",
                                      tag="rstd")
                    nc.scalar.activation(rstd[:], var[:], AF.Sqrt,
                                         bias=epsr[:])
                    nc.vector.reciprocal(out=rstd[:], in_=rstd[:])
                    rstdb = row_p.tile([1, 512], BF16, name=f"rstdb{tg}",
                                       tag="rstdb")
                    nc.vector.tensor_copy(out=rstdb[:], in_=rstd[:])
                    nmr = row_p.tile([1, 512], BF16, name=f"nmr{tg}",
                                     tag="nmr")
                    nc.vector.scalar_tensor_tensor(
                        out=nmr[:], in0=mu[:], scalar=-1.0, in1=rstd[:],
                        op0=ALU.mult, op1=ALU.mult)
                    pbc_r = ps_bc.tile([128, 512], F32, name=f"pbcr{tg}",
                                       tag="ps_bc")
                    nc.tensor.matmul(pbc_r[:], ones_row[:], rstdb[:],
                                     start=True, stop=True)
                    pbc_m = ps_bc.tile([128, 512], F32, name=f"pbcm{tg}",
                                       tag="ps_bc")
                    nc.tensor.matmul(pbc_m[:], ones_row[:], nmr[:],
                                     start=True, stop=True)

                    # x1T = (h1*rstd + (-mu*rstd)) * g1 + b1
                    for dc in range(DCH):
                        u = atmp_p.tile([128, 512], F32,
                                        name=f"u{tg}_{dc}", tag="u")
                        nc.vector.scalar_tensor_tensor(
                            out=u[:], in0=h1[:, dc, :],
                            scalar=g1_s[:, dc:dc + 1], in1=pbc_r[:],
                            op0=ALU.mult, op1=ALU.mult)
                        nc.vector.scalar_tensor_tensor(
                            out=u[:], in0=pbc_m[:],
                            scalar=g1_s[:, dc:dc + 1], in1=u[:],
                            op0=ALU.mult, op1=ALU.add)
                        nc.scalar.activation(
                            x1T[:, dc, ts(tg, 512)], u[:], AF.Identity,
                            bias=b1_s[:, dc:dc + 1])

        p12.close()  # kv/ksum accumulators dead after P2
        xres.close()  # x dead after P2

        # ---------------- P3: SwiGLU FFN + LN2, 2 passes of 1024 tokens ---
        NPASS = 2
        TGP = NTG // NPASS  # token groups per pass
        with ExitStack() as p3:
            ffn_p = p3.enter_context(tc.tile_pool(name="ffn", bufs=1))
            wgu_p = p3.enter_context(tc.tile_pool(name="wgu", bufs=2))
            wd_p = p3.enter_context(tc.tile_pool(name="wdp", bufs=1))

            # wd resident for all of P3 (44KB/partition), loaded during
            # pass-0 gate/up on the gpsimd queue
            wd_s = wd_p.tile([128, HCH, D], BF16, name="wd_s")
            for hh in range(2):
                nc.gpsimd.dma_start(out=wd_s[:, ts(hh, HCH // 2), :],
                                    in_=wd[:, ts(hh, HCH // 2), :])

            for psi in range(NPASS):
                ffn_t = ffn_p.tile([128, HCH, TGP * 512], BF16,
                                   name=f"ffn{psi}", tag="ffn")

                with ExitStack() as gsc:
                    ps_g = gsc.enter_context(
                        tc.tile_pool(name="ps_g", bufs=2, space="PSUM"))
                    ps_u = gsc.enter_context(
                        tc.tile_pool(name="ps_u", bufs=2, space="PSUM"))
                    sg_p = gsc.enter_context(tc.tile_pool(name="sg", bufs=3))
                    for hb in range(HBLK):
                        wg_t = wgu_p.tile([128, DCH, 256], BF16,
                                          name=f"wg{psi}_{hb}", tag="wg")
                        nc.scalar.dma_start(out=wg_t[:], in_=wg[:, hb])
                        wu_t = wgu_p.tile([128, DCH, 256], BF16,
                                          name=f"wu{psi}_{hb}", tag="wu")
                        nc.scalar.dma_start(out=wu_t[:], in_=wu[:, hb])
                        for hl in range(2):
                            hc = hb * 2 + hl
                            for tgh in range(TGP):
                                tg = psi * TGP + tgh
                                tsl = ts(tg, 512)
                                fsl = ts(tgh, 512)
                                psg = ps_g.tile([128, 512], F32,
                                                name=f"pg{psi}_{hc}_{tgh}",
                                                tag="ps_g")
                                for dc in range(DCH):
                                    nc.tensor.matmul(
                                        psg[:], wg_t[:, dc, ts(hl, 128)],
                                        x1T[:, dc, tsl],
                                        start=(dc == 0), stop=(dc == DCH - 1))
                                psu = ps_u.tile([128, 512], F32,
                                                name=f"pu{psi}_{hc}_{tgh}",
                                                tag="ps_u")
                                for dc in range(DCH):
                                    nc.tensor.matmul(
                                        psu[:], wu_t[:, dc, ts(hl, 128)],
                                        x1T[:, dc, tsl],
                                        start=(dc == 0), stop=(dc == DCH - 1))
                                sg = sg_p.tile([128, 512], F32,
                                               name=f"sg{psi}_{hc}_{tgh}",
                                               tag="sg")
                                nc.scalar.activation(
                                    sg[:], psg[:], AF.Silu,
                                    bias=bg_s[:, hc:hc + 1])
                                nc.vector.scalar_tensor_tensor(
                                    out=ffn_t[:, hc, fsl], in0=psu[:],
                                    scalar=bu_s[:, hc:hc + 1], in1=sg[:],
                                    op0=ALU.add, op1=ALU.mult)

                with ExitStack() as dsc:
                    h2_p = dsc.enter_context(tc.tile_pool(name="h2", bufs=1))
                    sq2_p = dsc.enter_context(tc.tile_pool(name="sq2",
                                                           bufs=1))
                    row2_p = dsc.enter_context(tc.tile_pool(name="rows2",
                                                            bufs=1))
                    ot_p = dsc.enter_context(tc.tile_pool(name="otp",
                                                          bufs=2))
                    otmp_p = dsc.enter_context(tc.tile_pool(name="otmp",
                                                            bufs=2))
                    ps_dn = dsc.enter_context(
                        tc.tile_pool(name="ps_dn", bufs=4, space="PSUM"))
                    ps_row2 = dsc.enter_context(
                        tc.tile_pool(name="ps_row2", bufs=2, space="PSUM"))
                    ps_bc2 = dsc.enter_context(
                        tc.tile_pool(name="ps_bc2", bufs=2, space="PSUM"))

                    for tgh in range(TGP):
                        tg = psi * TGP + tgh
                        tsl = ts(tg, 512)
                        fsl = ts(tgh, 512)
                        h2 = h2_p.tile([128, DCH, 512], BF16,
                                       name=f"h2_{psi}_{tgh}", tag="h2")
                        sq2 = sq2_p.tile([128, DCH, 512], BF16,
                                         name=f"sq2_{psi}_{tgh}", tag="sq2")
                        for dhalf in range(2):
                            psds = {}
                            for dq in range(4):
                                dc = dhalf * 4 + dq
                                psds[dc] = ps_dn.tile(
                                    [128, 512], F32,
                                    name=f"pd{psi}_{tgh}_{dc}", tag="ps_dn")
                            for hc in range(HCH):
                                for dq in range(4):
                                    dc = dhalf * 4 + dq
                                    nc.tensor.matmul(
                                        psds[dc][:],
                                        wd_s[:, hc, ts(dc, 128)],
                                        ffn_t[:, hc, fsl],
                                        start=(hc == 0), stop=(hc == HCH - 1))
                            for dq in range(4):
                                dc = dhalf * 4 + dq
                                nc.vector.scalar_tensor_tensor(
                                    out=h2[:, dc, :], in0=psds[dc][:],
                                    scalar=bd_s[:, dc:dc + 1],
                                    in1=x1T[:, dc, tsl],
                                    op0=ALU.add, op1=ALU.add)
                                nc.scalar.activation(
                                    sq2[:, dc, :], h2[:, dc, :], AF.Square)

                        # LN2 stats + affine + store
                        psm = ps_row2.tile([1, 512], F32,
                                           name=f"psm2_{psi}_{tgh}",
                                           tag="ps_row2")
                        for dc in range(DCH):
                            nc.tensor.matmul(psm[:], ones_col[:],
                                             h2[:, dc, :],
                                             start=(dc == 0),
                                             stop=(dc == DCH - 1))
                        pss = ps_row2.tile([1, 512], F32,
                                           name=f"pss2_{psi}_{tgh}",
                                           tag="ps_row2")
                        for dc in range(DCH):
                            nc.tensor.matmul(pss[:], ones_col[:],
                                             sq2[:, dc, :],
                                             start=(dc == 0),
                                             stop=(dc == DCH - 1))
                        mu = row2_p.tile([1, 512], F32,
                                         name=f"mu2_{psi}_{tgh}", tag="mu2")
                        nc.vector.tensor_scalar_mul(out=mu[:], in0=psm[:],
                                                    scalar1=1.0 / D)
                        ex2 = row2_p.tile([1, 512], F32,
                                          name=f"ex22_{psi}_{tgh}",
                                          tag="ex22")
                        nc.vector.tensor_scalar_mul(out=ex2[:], in0=pss[:],
                                                    scalar1=1.0 / D)
                        var = row2_p.tile([1, 512], F32,
                                          name=f"var2_{psi}_{tgh}",
                                          tag="var2")
                        nc.vector.tensor_tensor(out=var[:], in0=mu[:],
                                                in1=mu[:], op=ALU.mult)
                        nc.vector.tensor_tensor(out=var[:], in0=ex2[:],
                                                in1=var[:], op=ALU.subtract)
                        rstd = row2_p.tile([1, 512], F32,
                                           name=f"rstd2_{psi}_{tgh}",
                                           tag="rstd2")
                        nc.scalar.activation(rstd[:], var[:], AF.Sqrt,
                                             bias=epsr[:])
                        nc.vector.reciprocal(out=rstd[:], in_=rstd[:])
                        rstdb = row2_p.tile([1, 512], BF16,
                                            name=f"rstdb2_{psi}_{tgh}",
                                            tag="rstdb2")
                        nc.vector.tensor_copy(out=rstdb[:], in_=rstd[:])
                        nmr = row2_p.tile([1, 512], BF16,
                                          name=f"nmr2_{psi}_{tgh}",
                                          tag="nmr2")
                        nc.vector.scalar_tensor_tensor(
                            out=nmr[:], in0=mu[:], scalar=-1.0, in1=rstd[:],
                            op0=ALU.mult, op1=ALU.mult)
                        pbc_r = ps_bc2.tile([128, 512], F32,
                                            name=f"pbcr2_{psi}_{tgh}",
                                            tag="ps_bc2")
                        nc.tensor.matmul(pbc_r[:], ones_row[:], rstdb[:],
                                         start=True, stop=True)
                        pbc_m = ps_bc2.tile([128, 512], F32,
                                            name=f"pbcm2_{psi}_{tgh}",
                                            tag="ps_bc2")
                        nc.tensor.matmul(pbc_m[:], ones_row[:], nmr[:],
                                         start=True, stop=True)

                        o_t = ot_p.tile([128, DCH, 512], BF16,
                                        name=f"o_{psi}_{tgh}", tag="ot")
                        for dc in range(DCH):
                            u = otmp_p.tile([128, 512], F32,
                                            name=f"ou{psi}_{tgh}_{dc}",
                                            tag="ou")
                            nc.vector.scalar_tensor_tensor(
                                out=u[:], in0=h2[:, dc, :],
                                scalar=g2_s[:, dc:dc + 1], in1=pbc_r[:],
                                op0=ALU.mult, op1=ALU.mult)
                            nc.vector.scalar_tensor_tensor(
                                out=u[:], in0=pbc_m[:],
                                scalar=g2_s[:, dc:dc + 1], in1=u[:],
                                op0=ALU.mult, op1=ALU.add)
                            nc.scalar.activation(
                                o_t[:, dc, :], u[:], AF.Identity,
                                bias=b2_s[:, dc:dc + 1])
                        nc.sync.dma_start(out=out_v[:, :, tsl], in_=o_t[:])

    nc.compile()
    return nc, input_names


# ---------------------------------------------------------------------------
# Host-side wrapper
# ---------------------------------------------------------------------------

B, S, D_MODEL, D_FF = 4, 4096, 1024, 4096
FFN_H = int(2 * D_FF / 3)  # 2730

_cache = {}
LAST_RESULTS = None
BF16_NP = ml_dtypes.bfloat16


def _get_program():
    if "prog" not in _cache:
        _cache["prog"] = build_program()
    return _cache["prog"]


def _prep_shared(Wqkv, bqkv, Wg, bg, Wu, bu, Wd, bd, g1, b1, g2, b2):
    f = np.float32

    def chunk_in(w):  # [1024, N] -> [128, 8, N] with d = c*128+p
        return np.ascontiguousarray(
            w.reshape(8, 128, -1).transpose(1, 0, 2)).astype(BF16_NP)

    Wqkv = np.asarray(Wqkv, f)
    sh = {}
    sh["wq"] = chunk_in(Wqkv[:, 0:1024])
    sh["wk"] = chunk_in(Wqkv[:, 1024:2048])
    sh["wv"] = chunk_in(Wqkv[:, 2048:3072])
    bqkv = np.asarray(bqkv, f)
    sh["bq_col"] = np.ascontiguousarray(
        bqkv[0:1024].reshape(8, 128).T).astype(f)
    sh["bk_row"] = bqkv[1024:2048].reshape(1, 1024).astype(BF16_NP)
    sh["bv_row"] = bqkv[2048:3072].reshape(1, 1024).astype(BF16_NP)

    wg_p = np.zeros((1024, H_PAD), f)
    wg_p[:, :FFN_H] = np.asarray(Wg, f)
    wu_p = np.zeros((1024, H_PAD), f)
    wu_p[:, :FFN_H] = np.asarray(Wu, f)

    def chunk_gu(w):  # [1024, 2816] -> [128, 11, 8, 256]
        w = w.reshape(8, 128, HBLK, 256)  # (c, p, hb, hsub)
        return np.ascontiguousarray(w.transpose(1, 2, 0, 3)).astype(BF16_NP)

    sh["wg"] = chunk_gu(wg_p)
    sh["wu"] = chunk_gu(wu_p)

    def col_pad(b, n):
        bp = np.zeros((n,), f)
        bp[:len(b)] = np.asarray(b, f)
        return np.ascontiguousarray(bp.reshape(n // 128, 128).T).astype(f)

    sh["bg_col"] = col_pad(np.asarray(bg, f), H_PAD)
    sh["bu_col"] = col_pad(np.asarray(bu, f), H_PAD)

    wd_p = np.zeros((H_PAD, 1024), f)
    wd_p[:FFN_H, :] = np.asarray(Wd, f)
    sh["wd"] = np.ascontiguousarray(
        wd_p.reshape(HCH, 128, 1024).transpose(1, 0, 2)).astype(BF16_NP)
    sh["bd_col"] = col_pad(np.asarray(bd, f), 1024)
    sh["g1_col"] = col_pad(np.asarray(g1, f), 1024)
    sh["b1_col"] = col_pad(np.asarray(b1, f), 1024)
    sh["g2_col"] = col_pad(np.asarray(g2, f), 1024)
    sh["b2_col"] = col_pad(np.asarray(b2, f), 1024)
    return sh


def make_in_maps(x, Wqkv, bqkv, Wg, bg, Wu, bu, Wd, bd, g1, b1, g2, b2):
    x = np.asarray(x, np.float32)
    sh = _prep_shared(Wqkv, bqkv, Wg, bg, Wu, bu, Wd, bd, g1, b1, g2, b2)
    in_maps = []
    for c in range(8):
        b, h = c // 2, c % 2
        m = dict(sh)
        xs = x[b, h * 2048:(h + 1) * 2048]  # [2048, 1024]
        # xh[p, c, t] = x[t, c*128+p]
        m["xh"] = np.ascontiguousarray(
            xs.reshape(2048, 8, 128).transpose(2, 1, 0)).astype(BF16_NP)
        in_maps.append(m)
    return in_maps


def kernel(x, Wqkv, bqkv, Wg, bg, Wu, bu, Wd, bd, g1, b1, g2, b2):
    global LAST_RESULTS
    from concourse import bass_utils

    nc, _names = _get_program()
    in_maps = make_in_maps(x, Wqkv, bqkv, Wg, bg, Wu, bu, Wd, bd,
                           g1, b1, g2, b2)
    res = bass_utils.run_bass_kernel_spmd(nc, in_maps, core_ids=list(range(8)))
    LAST_RESULTS = res
    out = np.empty((B, S, D_MODEL), np.float32)
    for c in range(8):
        b, h = c // 2, c % 2
        # device output is [D, T_OWN] with row index = c*128+p
        out[b, h * 2048:(h + 1) * 2048] = (
            res.results[c]["out"].astype(np.float32).T)
    return out


# revision 25
# speedup vs baseline: 1.4229x; 1.3271x over previous
"""Trainium2 Bass kernel for nn_Block_54219667145535 (linear-attention block).

v3: transposed (feature-major) pipeline, bf16 attention + fp8 DoubleRow FFN.

Sharding: 8 cores, 2 per batch (B=4). Each core computes k/v projections +
partial [D,D] kv state over its own 2048 tokens, pair-AllReduces the packed
(kv|ksum) buffer (single contiguous 16.4KB/partition descriptor), and hides
the collective under the q projection. Everything downstream stays in
[d-partition, token] orientation (no PE transposes, no x1 DRAM round-trip);
LN stats are ones-matmuls, per-token scales are rank-1 broadcast matmuls.
The SwiGLU FFN runs in fp8e4m3 with DoubleRow (K_eff=256), weights scaled by
256 on the host and descaled in the PSUM evacuation. Host pre-transposes x
and all weights and re-transposes the output.
"""

import os
import sys
from contextlib import ExitStack

import numpy as np


def _ensure_paths():
    for p in ("/opt/trn_rl_repo", "/root/.axon_site/_ro/trn_rl_repo"):
        if os.path.isdir(p) and p not in sys.path:
            sys.path.insert(0, p)
    try:
        import concourse.bass  # noqa: F401
    except ImportError as e:  # pragma: no cover
        raise ImportError(f"concourse not importable: {e}")


_ensure_paths()

import ml_dtypes  # noqa: E402

import concourse.bacc as bacc  # noqa: E402
import concourse.tile as tile  # noqa: E402
from concourse import mybir  # noqa: E402
from concourse.bass import ts  # noqa: E402

F32 = mybir.dt.float32
BF16 = mybir.dt.bfloat16
FP8 = mybir.dt.float8e4
AF = mybir.ActivationFunctionType
ALU = mybir.AluOpType
DR = mybir.MatmulPerfMode.DoubleRow

D = 1024
DCH = 8          # d chunks of 128
H_PAD = 2816
HCH = 22         # h chunks of 128
HBLK = 11        # h blocks of 256 (weight streaming granularity)
LN_EPS = 1e-5
ATTN_EPS = 1e-6
KV_W = DCH * D   # 8192 bf16 kv values per partition
PK_W = KV_W + DCH  # + 8 ksum values
WS = 256.0       # fp8 weight scale
WSI = 1.0 / WS


def build_program(T_OWN=2048, n_cores=8):
    """Per-core Bass/Tile program. Pair (2b, 2b+1) handles batch b."""
    NBLK = T_OWN // 512   # P1 token blocks
    NTG = T_OWN // 512    # P2/P3 token groups
    GROUPS = [[c, c + 1] for c in range(0, n_cores, 2)]

    nc = bacc.Bacc(
        "TRN2",
        target_bir_lowering=False,
        debug=False,
        enable_asserts=False,
        num_devices=8,
        num_swdge_queues=4,
    )

    # ---- I/O (host supplies pre-transposed / pre-chunked layouts) ----
    xh = nc.dram_tensor("xh", [128, DCH, T_OWN], BF16, kind="ExternalInput").ap()
    wk = nc.dram_tensor("wk", [128, DCH, D], BF16, kind="ExternalInput").ap()
    wv = nc.dram_tensor("wv", [128, DCH, D], BF16, kind="ExternalInput").ap()
    wq = nc.dram_tensor("wq", [128, DCH, D], BF16, kind="ExternalInput").ap()
    bk_row = nc.dram_tensor("bk_row", [1, D], BF16, kind="ExternalInput").ap()
    bv_row = nc.dram_tensor("bv_row", [1, D], BF16, kind="ExternalInput").ap()
    bq_col = nc.dram_tensor("bq_col", [128, DCH], F32, kind="ExternalInput").ap()
    # wg/wu: [p, hb, c, hsub] fp8 (x256); lhsT chunk = [:, hb, 2j:2j+2, hsl]
    wg = nc.dram_tensor("wg", [128, HBLK, DCH, 256], FP8,
                        kind="ExternalInput").ap()
    wu = nc.dram_tensor("wu", [128, HBLK, DCH, 256], FP8,
                        kind="ExternalInput").ap()
    bg_col = nc.dram_tensor("bg_col", [128, HCH], F32, kind="ExternalInput").ap()
    bu_col = nc.dram_tensor("bu_col", [128, HCH], F32, kind="ExternalInput").ap()
    # wd[p, hc, d] fp8 (x256)
    wd = nc.dram_tensor("wd", [128, HCH, D], FP8, kind="ExternalInput").ap()
    bd_col = nc.dram_tensor("bd_col", [128, DCH], F32, kind="ExternalInput").ap()
    g1_col = nc.dram_tensor("g1_col", [128, DCH], F32, kind="ExternalInput").ap()
    b1_col = nc.dram_tensor("b1_col", [128, DCH], F32, kind="ExternalInput").ap()
    g2_col = nc.dram_tensor("g2_col", [128, DCH], F32, kind="ExternalInput").ap()
    b2_col = nc.dram_tensor("b2_col", [128, DCH], F32, kind="ExternalInput").ap()
    out = nc.dram_tensor("out", [D, T_OWN], BF16, kind="ExternalOutput").ap()
    out_v = out.rearrange("(c p) t -> p c t", p=128)

    input_names = [
        "xh", "wk", "wv", "wq", "bk_row", "bv_row", "bq_col", "wg", "wu",
        "bg_col", "bu_col", "wd", "bd_col", "g1_col", "b1_col", "g2_col",
        "b2_col",
    ]

    def bcast_row(row_ap, parts=128):
        import concourse.bass as bass
        return bass.AP(
            tensor=row_ap.tensor,
            offset=row_ap.offset,
            ap=[[0, parts]] + [list(d) for d in row_ap.ap[1:]],
        )

    with tile.TileContext(nc) as tc, ExitStack() as top:
        dram = top.enter_context(tc.tile_pool(name="dram", bufs=1, space="DRAM"))
        kv_pack = dram.tile([128, PK_W], BF16, name="kv_pack")
        kv_out = dram.tile([128, PK_W], BF16, name="kv_out")

        consts = top.enter_context(tc.tile_pool(name="consts", bufs=1))
        ones_col = consts.tile([128, 1], BF16, name="ones_col")
        nc.vector.memset(ones_col[:], 1.0)
        ones_row = consts.tile([1, 128], BF16, name="ones_row")
        nc.vector.memset(ones_row[:], 1.0)
        one1 = consts.tile([1, 1], F32, name="one1")
        nc.vector.memset(one1[:], 1.0)
        epsr = consts.tile([1, 1], F32, name="epsr")
        nc.vector.memset(epsr[:], LN_EPS)
        bq_s = consts.tile([128, DCH], F32, name="bq_s")
        nc.sync.dma_start(out=bq_s[:], in_=bq_col)
        nbq_s = consts.tile([128, DCH], F32, name="nbq_s")
        nc.vector.tensor_scalar_mul(out=nbq_s[:], in0=bq_s[:], scalar1=-1.0)
        bg_s = consts.tile([128, HCH], F32, name="bg_s")
        nc.sync.dma_start(out=bg_s[:], in_=bg_col)
        bu_s = consts.tile([128, HCH], F32, name="bu_s")
        nc.sync.dma_start(out=bu_s[:], in_=bu_col)
        bd_s = consts.tile([128, DCH], F32, name="bd_s")
        nc.sync.dma_start(out=bd_s[:], in_=bd_col)
        g1_s = consts.tile([128, DCH], F32, name="g1_s")
        nc.sync.dma_start(out=g1_s[:], in_=g1_col)
        b1_s = consts.tile([128, DCH], F32, name="b1_s")
        nc.sync.dma_start(out=b1_s[:], in_=b1_col)
        g2_s = consts.tile([128, DCH], F32, name="g2_s")
        nc.sync.dma_start(out=g2_s[:], in_=g2_col)
        b2_s = consts.tile([128, DCH], F32, name="b2_s")
        nc.sync.dma_start(out=b2_s[:], in_=b2_col)

        # x1 (LN1 output): bf16 for the h2 residual; fp8 copy made in P3
        x1res = top.enter_context(tc.tile_pool(name="x1res", bufs=1))
        x1T = x1res.tile([128, DCH, T_OWN], BF16, name="x1T")

        # x resident through P1+P2 (32KB/partition), freed before P3
        xres = ExitStack()
        xres_p = xres.enter_context(tc.tile_pool(name="xres", bufs=1))
        x_sb = xres_p.tile([128, DCH, T_OWN], BF16, name="x_sb")
        for blk in range(NBLK):
            nc.sync.dma_start(out=x_sb[:, :, ts(blk, 512)],
                              in_=xh[:, :, ts(blk, 512)])

        # kv+ksum accumulator, also the collective staging target
        p12 = ExitStack()
        accs = p12.enter_context(tc.tile_pool(name="accs", bufs=1))
        kv_sb = accs.tile([128, PK_W], BF16, name="kv_sb")
        kv_v = kv_sb[:][:, 0:KV_W].rearrange("p (c e) -> p c e", c=DCH)
        ks_v = kv_sb[:][:, KV_W:PK_W].rearrange("p (c e) -> p c e", c=DCH)

        # qp output pool opened early (before wqres) to keep pool LIFO order
        qp_es = ExitStack()
        qp_p = qp_es.enter_context(tc.tile_pool(name="qp", bufs=1))
        qpT = qp_p.tile([128, DCH, T_OWN], BF16, name="qpT")

        # wq preloaded up-front so q-proj starts the instant P1 ends
        # (LIFO: closed right after the q-proj scope)
        wqres = ExitStack()
        wq_pool = wqres.enter_context(tc.tile_pool(name="wqres", bufs=1))
        wq_s = wq_pool.tile([128, DCH, D], BF16, name="wq_s")

        # ---------------- P1: k/v proj + kv/ksum over own tokens ----------
        with ExitStack() as p1:
            wkv_p = p1.enter_context(tc.tile_pool(name="wkv", bufs=1))
            c1_p = p1.enter_context(tc.tile_pool(name="c1", bufs=1))
            kpv_p = p1.enter_context(tc.tile_pool(name="kpv", bufs=1))
            tmp_p = p1.enter_context(tc.tile_pool(name="p1tmp", bufs=2))
            ksr_p = p1.enter_context(tc.tile_pool(name="ksr", bufs=1))
            ps_proj = p1.enter_context(
                tc.tile_pool(name="ps_proj", bufs=3, space="PSUM"))
            ps_kv = p1.enter_context(
                tc.tile_pool(name="ps_kv", bufs=3, space="PSUM"))
            ps_ks = p1.enter_context(
                tc.tile_pool(name="ps_ks", bufs=2, space="PSUM"))

            wk_s = wkv_p.tile([128, DCH, D], BF16, name="wk_s")
            wv_s = wkv_p.tile([128, DCH, D], BF16, name="wv_s")
            for half in range(2):
                nc.scalar.dma_start(out=wk_s[:, :, ts(half, 512)],
                                    in_=wk[:, :, ts(half, 512)])
            for half in range(2):
                nc.scalar.dma_start(out=wv_s[:, :, ts(half, 512)],
                                    in_=wv[:, :, ts(half, 512)])
            for half in range(2):
                nc.scalar.dma_start(out=wq_s[:, :, ts(half, 512)],
                                    in_=wq[:, :, ts(half, 512)])
            bkb = c1_p.tile([128, D], BF16, name="bkb")
            nc.sync.dma_start(out=bkb[:], in_=bcast_row(bk_row))
            bvb = c1_p.tile([128, D], BF16, name="bvb")
            nc.sync.dma_start(out=bvb[:], in_=bcast_row(bv_row))

            ks_ps = [ps_ks.tile([1, 512], F32, name=f"ksps{e}", tag="ps_ks")
                     for e in range(2)]

            for blk in range(NBLK):
                kp_blk = kpv_p.tile([128, 4, D], BF16, name=f"kp{blk}",
                                    tag="kp")
                v_blk = kpv_p.tile([128, 4, D], BF16, name=f"v{blk}", tag="v")

                for t4 in range(4):
                    tok0 = blk * 512 + t4 * 128
                    for which in range(2):  # 0 = k, 1 = v
                        w_s = wk_s if which == 0 else wv_s
                        for half in range(2):
                            gsl = ts(half, 512)
                            ps = ps_proj.tile(
                                [128, 512], F32,
                                name=f"pp{blk}_{t4}_{which}_{half}",
                                tag="ps_proj")
                            for dc in range(DCH):
                                nc.tensor.matmul(
                                    ps[:], x_sb[:, dc, tok0:tok0 + 128],
                                    w_s[:, dc, gsl],
                                    start=(dc == 0), stop=(dc == DCH - 1))
                            if which == 0:
                                # kp = elu(k+bk)+1 = exp(-r) + kb + r,
                                # r = relu(-kb), kb = k + bk
                                kb = tmp_p.tile([128, 512], F32,
                                                name=f"kb{blk}_{t4}_{half}",
                                                tag="kb")
                                nc.vector.tensor_tensor(
                                    out=kb[:], in0=ps[:], in1=bkb[:, gsl],
                                    op=ALU.add)
                                r = tmp_p.tile([128, 512], F32,
                                               name=f"r{blk}_{t4}_{half}",
                                               tag="r")
                                nc.scalar.activation(r[:], kb[:], AF.Relu,
                                                     scale=-1.0)
                                e = tmp_p.tile([128, 512], F32,
                                               name=f"e{blk}_{t4}_{half}",
                                               tag="e")
                                nc.scalar.activation(e[:], r[:], AF.Exp,
                                                     scale=-1.0)
                                nc.vector.tensor_tensor(
                                    out=kb[:], in0=kb[:], in1=r[:], op=ALU.add)
                                nc.vector.tensor_tensor(
                                    out=kp_blk[:, t4, gsl], in0=kb[:],
                                    in1=e[:], op=ALU.add)
                            else:
                                nc.vector.tensor_tensor(
                                    out=v_blk[:, t4, gsl], in0=ps[:],
                                    in1=bvb[:, gsl], op=ALU.add)

                    # ksum += ones^T @ kp for this t4 (both e halves)
                    for ec in range(2):
                        nc.tensor.matmul(
                            ks_ps[ec][:], ones_col[:],
                            kp_blk[:, t4, ts(ec, 512)],
                            start=(blk == 0 and t4 == 0),
                            stop=(blk == NBLK - 1 and t4 == 3))

                # kv[dc, e-half] += kp_blk^T @ v_blk
                for dc in range(DCH):
                    dsl = ts(dc, 128)
                    for ec in range(2):
                        esl = ts(ec, 512)
                        pkv = ps_kv.tile([128, 512], F32,
                                         name=f"pkv{blk}_{dc}_{ec}",
                                         tag="ps_kv")
                        for t4 in range(4):
                            nc.tensor.matmul(
                                pkv[:], kp_blk[:, t4, dsl],
                                v_blk[:, t4, esl],
                                start=(t4 == 0), stop=(t4 == 3))
                        if blk == 0:
                            nc.vector.tensor_copy(
                                out=kv_v[:, dc, esl], in_=pkv[:])
                        else:
                            nc.vector.tensor_tensor(
                                out=kv_v[:, dc, esl], in0=kv_v[:, dc, esl],
                                in1=pkv[:], op=ALU.add)

            # ksum rows [1,1024] -> per-partition cols kv_sb[:, KV_W:]
            ks_row = ksr_p.tile([1, D], F32, name="ks_row")
            for ec in range(2):
                nc.scalar.copy(out=ks_row[:, ts(ec, 512)], in_=ks_ps[ec][:])
            for dc in range(DCH):
                ptk = ps_proj.tile([128, 1], F32, name=f"ptk{dc}",
                                   tag="ps_proj")
                nc.tensor.transpose(ptk[:], ks_row[:, ts(dc, 128)],
                                    one1[:])
                nc.scalar.copy(out=ks_v[:, dc, :], in_=ptk[:])

        # ---- pair AllReduce of packed (kv | ksum), single descriptor ----
        nc.sync.dma_start(out=kv_pack[:], in_=kv_sb[:])
        nc.gpsimd.collective_compute(
            "AllReduce", ALU.add,
            ins=[kv_pack[:]], outs=[kv_out[:]], replica_groups=GROUPS)
        nc.gpsimd.dma_start(out=kv_sb[:], in_=kv_out[:])

        # ---------------- P2: q proj (hides collective), then attn + LN1 --
        with ExitStack() as p2:
            with ExitStack() as qsc:
                qtmp_p = qsc.enter_context(tc.tile_pool(name="qtmp", bufs=3))
                ps_q = qsc.enter_context(
                    tc.tile_pool(name="ps_q", bufs=3, space="PSUM"))
                for tg in range(NTG):
                    tsl = ts(tg, 512)
                    for qc in range(DCH):
                        ps = ps_q.tile([128, 512], F32, name=f"pq{tg}_{qc}",
                                       tag="ps_q")
                        for dc in range(DCH):
                            nc.tensor.matmul(
                                ps[:], wq_s[:, dc, ts(qc, 128)],
                                x_sb[:, dc, tsl],
                                start=(dc == 0), stop=(dc == DCH - 1))
                        # qp = elu(q+bq)+1; per-partition bias
                        r = qtmp_p.tile([128, 512], F32, name=f"qr{tg}_{qc}",
                                        tag="qr")
                        nc.scalar.activation(r[:], ps[:], AF.Relu,
                                             scale=-1.0,
                                             bias=nbq_s[:, qc:qc + 1])
                        e = qtmp_p.tile([128, 512], F32, name=f"qe{tg}_{qc}",
                                        tag="qe")
                        nc.scalar.activation(e[:], r[:], AF.Exp, scale=-1.0)
                        t = qtmp_p.tile([128, 512], F32, name=f"qt{tg}_{qc}",
                                        tag="qt")
                        nc.vector.scalar_tensor_tensor(
                            out=t[:], in0=ps[:], scalar=bq_s[:, qc:qc + 1],
                            in1=r[:], op0=ALU.add, op1=ALU.add)
                        nc.vector.tensor_tensor(
                            out=qpT[:, qc, tsl], in0=t[:], in1=e[:],
                            op=ALU.add)
            wqres.close()

            with ExitStack() as asc:
                h1_p = asc.enter_context(tc.tile_pool(name="h1", bufs=2))
                sq_p = asc.enter_context(tc.tile_pool(name="sq", bufs=2))
                row_p = asc.enter_context(tc.tile_pool(name="rows", bufs=1))
                atmp_p = asc.enter_context(tc.tile_pool(name="atmp", bufs=3))
                dbc_p = asc.enter_context(tc.tile_pool(name="dbc", bufs=2))
                ps_row = asc.enter_context(
                    tc.tile_pool(name="ps_row", bufs=3, space="PSUM"))
                ps_num = asc.enter_context(
                    tc.tile_pool(name="ps_num", bufs=2, space="PSUM"))
                ps_bc = asc.enter_context(
                    tc.tile_pool(name="ps_bc", bufs=3, space="PSUM"))

                hs = {}

                def stage_a(tg):
                    tsl = ts(tg, 512)
                    # den row = ksum^T qp + eps, then reciprocal
                    pdn = ps_row.tile([1, 512], F32, name=f"pdn{tg}",
                                      tag="ps_row")
                    for dc in range(DCH):
                        nc.tensor.matmul(
                            pdn[:], ks_v[:, dc, :], qpT[:, dc, tsl],
                            start=(dc == 0), stop=(dc == DCH - 1))
                    dnr = row_p.tile([1, 512], F32, name=f"dnr{tg}",
                                     tag="dnr")
                    nc.vector.tensor_scalar_add(
                        out=dnr[:], in0=pdn[:], scalar1=ATTN_EPS)
                    nc.vector.reciprocal(out=dnr[:], in_=dnr[:])
                    dnb = row_p.tile([1, 512], BF16, name=f"dnb{tg}",
                                     tag="dnb")
                    nc.vector.tensor_copy(out=dnb[:], in_=dnr[:])
                    pbc_d = ps_bc.tile([128, 512], F32, name=f"pbcd{tg}",
                                       tag="ps_bc")
                    nc.tensor.matmul(pbc_d[:], ones_row[:], dnb[:],
                                     start=True, stop=True)
                    dbc = dbc_p.tile([128, 512], F32, name=f"dbc{tg}",
                                     tag="dbc")
                    nc.scalar.copy(out=dbc[:], in_=pbc_d[:])

                    h1 = h1_p.tile([128, DCH, 512], BF16, name=f"h1_{tg}",
                                   tag="h1")
                    sq = sq_p.tile([128, DCH, 512], BF16, name=f"sq_{tg}",
                                   tag="sq")
                    for ec in range(DCH):
                        pn = ps_num.tile([128, 512], F32,
                                         name=f"pn{tg}_{ec}", tag="ps_num")
                        for dc in range(DCH):
                            nc.tensor.matmul(
                                pn[:], kv_v[:, dc, ts(ec, 128)],
                                qpT[:, dc, tsl],
                                start=(dc == 0), stop=(dc == DCH - 1))
                        at = atmp_p.tile([128, 512], F32,
                                         name=f"at{tg}_{ec}", tag="at")
                        nc.vector.tensor_tensor(
                            out=at[:], in0=pn[:], in1=dbc[:], op=ALU.mult)
                        nc.vector.tensor_tensor(
                            out=h1[:, ec, :], in0=at[:],
                            in1=x_sb[:, ec, tsl], op=ALU.add)
                        nc.scalar.activation(sq[:, ec, :], h1[:, ec, :],
                                             AF.Square)
                    hs[tg] = (h1, sq)

                def stage_b(tg):
                    tsl = ts(tg, 512)
                    h1, sq = hs[tg]
                    psm = ps_row.tile([1, 512], F32, name=f"psm{tg}",
                                      tag="ps_row")
                    for dc in range(DCH):
                        nc.tensor.matmul(psm[:], ones_col[:], h1[:, dc, :],
                                         start=(dc == 0),
                                         stop=(dc == DCH - 1))
                    pss = ps_row.tile([1, 512], F32, name=f"pss{tg}",
                                      tag="ps_row")
                    for dc in range(DCH):
                        nc.tensor.matmul(pss[:], ones_col[:], sq[:, dc, :],
                                         start=(dc == 0),
                                         stop=(dc == DCH - 1))
                    mu = row_p.tile([1, 512], F32, name=f"mu{tg}", tag="mu")
                    nc.vector.tensor_scalar_mul(out=mu[:], in0=psm[:],
                                                scalar1=1.0 / D)
                    ex2 = row_p.tile([1, 512], F32, name=f"ex2{tg}",
                                     tag="ex2")
                    nc.vector.tensor_scalar_mul(out=ex2[:], in0=pss[:],
                                                scalar1=1.0 / D)
                    var = row_p.tile([1, 512], F32, name=f"var{tg}",
                                     tag="var")
                    nc.vector.tensor_tensor(out=var[:], in0=mu[:], in1=mu[:],
                                            op=ALU.mult)
                    nc.vector.tensor_tensor(out=var[:], in0=ex2[:],
                                            in1=var[:], op=ALU.subtract)
                    rstd = row_p.tile([1, 512], F32, name=f"rstd{tg}",
                                      tag="rstd")
                    nc.scalar.activation(rstd[:], var[:], AF.Sqrt,
                                         bias=epsr[:])
                    nc.vector.reciprocal(out=rstd[:], in_=rstd[:])
                    rstdb = row_p.tile([1, 512], BF16, name=f"rstdb{tg}",
                                       tag="rstdb")
                    nc.vector.tensor_copy(out=rstdb[:], in_=rstd[:])
                    nmr = row_p.tile([1, 512], BF16, name=f"nmr{tg}",
                                     tag="nmr")
                    nc.vector.scalar_tensor_tensor(
                        out=nmr[:], in0=mu[:], scalar=-1.0, in1=rstd[:],
                        op0=ALU.mult, op1=ALU.mult)
                    pbc_r = ps_bc.tile([128, 512], F32, name=f"pbcr{tg}",
                                       tag="ps_bc")
                    nc.tensor.matmul(pbc_r[:], ones_row[:], rstdb[:],
                                     start=True, stop=True)
                    pbc_m = ps_bc.tile([128, 512], F32, name=f"pbcm{tg}",
                                       tag="ps_bc")
                    nc.tensor.matmul(pbc_m[:], ones_row[:], nmr[:],
                                     start=True, stop=True)

                    # x1 = (h1*rstd + (-mu*rstd)) * g1 + b1  (bf16 + fp8)
                    for dc in range(DCH):
                        u = atmp_p.tile([128, 512], F32,
                                        name=f"u{tg}_{dc}", tag="u")
                        nc.vector.scalar_tensor_tensor(
                            out=u[:], in0=h1[:, dc, :],
                            scalar=g1_s[:, dc:dc + 1], in1=pbc_r[:],
                            op0=ALU.mult, op1=ALU.mult)
                        nc.vector.scalar_tensor_tensor(
                            out=u[:], in0=pbc_m[:],
                            scalar=g1_s[:, dc:dc + 1], in1=u[:],
                            op0=ALU.mult, op1=ALU.add)
                        nc.scalar.activation(
                            x1T[:, dc, tsl], u[:], AF.Identity,
                            bias=b1_s[:, dc:dc + 1])

                # software pipeline: stats of tg overlap num of tg+1
                stage_a(0)
                for tg in range(1, NTG):
                    stage_a(tg)
                    stage_b(tg - 1)
                stage_b(NTG - 1)

        qp_es.close()  # qp dead after attn
        p12.close()  # kv/ksum accumulators dead after P2
        xres.close()  # x dead after P2

        # ---------------- P3: fp8 DoubleRow SwiGLU FFN + LN2 --------------
        NPASS = 2
        TGP = NTG // NPASS
        with ExitStack() as p3:
            x1f8_p = p3.enter_context(tc.tile_pool(name="x1f8p", bufs=1))
            x1f8 = x1f8_p.tile([128, DCH, T_OWN], FP8, name="x1f8")
            for dc in range(DCH):
                nc.scalar.copy(out=x1f8[:, dc, :], in_=x1T[:, dc, :])

            ffn_p = p3.enter_context(tc.tile_pool(name="ffn", bufs=1))
            wgu_p = p3.enter_context(tc.tile_pool(name="wgu", bufs=2))
            wd_p = p3.enter_context(tc.tile_pool(name="wdp", bufs=1))

            # wd resident for all of P3 (22KB/partition fp8)
            wd_s = wd_p.tile([128, HCH, D], FP8, name="wd_s")
            for hh in range(2):
                nc.gpsimd.dma_start(out=wd_s[:, ts(hh, HCH // 2), :],
                                    in_=wd[:, ts(hh, HCH // 2), :])

            for psi in range(NPASS):
                ffn_t = ffn_p.tile([128, HCH, TGP * 512], FP8,
                                   name=f"ffn{psi}", tag="ffn")

                with ExitStack() as gsc:
                    ps_g = gsc.enter_context(
                        tc.tile_pool(name="ps_g", bufs=2, space="PSUM"))
                    ps_u = gsc.enter_context(
                        tc.tile_pool(name="ps_u", bufs=2, space="PSUM"))
                    sg_p = gsc.enter_context(tc.tile_pool(name="sg", bufs=3))
                    ut_p = gsc.enter_context(tc.tile_pool(name="ut", bufs=3))
                    for hb in range(HBLK):
                        wg_t = wgu_p.tile([128, DCH, 256], FP8,
                                          name=f"wg{psi}_{hb}", tag="wg")
                        nc.scalar.dma_start(out=wg_t[:], in_=wg[:, hb])
                        wu_t = wgu_p.tile([128, DCH, 256], FP8,
                                          name=f"wu{psi}_{hb}", tag="wu")
                        nc.scalar.dma_start(out=wu_t[:], in_=wu[:, hb])
                        for hl in range(2):
                            hc = hb * 2 + hl
                            hsl = ts(hl, 128)
                            for tgh in range(TGP):
                                tg = psi * TGP + tgh
                                tsl = ts(tg, 512)
                                fsl = ts(tgh, 512)
                                psg = ps_g.tile([128, 512], F32,
                                                name=f"pg{psi}_{hc}_{tgh}",
                                                tag="ps_g")
                                for jp in range(DCH // 2):
                                    nc.tensor.matmul(
                                        psg[:],
                                        wg_t[:, 2 * jp:2 * jp + 2, hsl],
                                        x1f8[:, 2 * jp:2 * jp + 2, tsl],
                                        start=(jp == 0),
                                        stop=(jp == DCH // 2 - 1),
                                        perf_mode=DR)
                                psu = ps_u.tile([128, 512], F32,
                                                name=f"pu{psi}_{hc}_{tgh}",
                                                tag="ps_u")
                                for jp in range(DCH // 2):
                                    nc.tensor.matmul(
                                        psu[:],
                                        wu_t[:, 2 * jp:2 * jp + 2, hsl],
                                        x1f8[:, 2 * jp:2 * jp + 2, tsl],
                                        start=(jp == 0),
                                        stop=(jp == DCH // 2 - 1),
                                        perf_mode=DR)
                                sg = sg_p.tile([128, 512], F32,
                                               name=f"sg{psi}_{hc}_{tgh}",
                                               tag="sg")
                                nc.scalar.activation(
                                    sg[:], psg[:], AF.Silu,
                                    bias=bg_s[:, hc:hc + 1], scale=WSI)
                                ut = ut_p.tile([128, 512], F32,
                                               name=f"ut{psi}_{hc}_{tgh}",
                                               tag="ut")
                                nc.vector.tensor_scalar(
                                    out=ut[:], in0=psu[:], scalar1=WSI,
                                    scalar2=bu_s[:, hc:hc + 1],
                                    op0=ALU.mult, op1=ALU.add)
                                nc.vector.tensor_tensor(
                                    out=ffn_t[:, hc, fsl], in0=ut[:],
                                    in1=sg[:], op=ALU.mult)

                with ExitStack() as dsc:
                    h2_p = dsc.enter_context(tc.tile_pool(name="h2", bufs=2))
                    sq2_p = dsc.enter_context(tc.tile_pool(name="sq2",
                                                           bufs=2))
                    row2_p = dsc.enter_context(tc.tile_pool(name="rows2",
                                                            bufs=1))
                    ot_p = dsc.enter_context(tc.tile_pool(name="otp",
                                                          bufs=2))
                    otmp_p = dsc.enter_context(tc.tile_pool(name="otmp",
                                                            bufs=2))
                    ps_dn = dsc.enter_context(
                        tc.tile_pool(name="ps_dn", bufs=4, space="PSUM"))
                    ps_row2 = dsc.enter_context(
                        tc.tile_pool(name="ps_row2", bufs=2, space="PSUM"))
                    ps_bc2 = dsc.enter_context(
                        tc.tile_pool(name="ps_bc2", bufs=2, space="PSUM"))

                    h2s = {}
                    for tgh in range(TGP):
                        tg = psi * TGP + tgh
                        tsl = ts(tg, 512)
                        fsl = ts(tgh, 512)
                        h2 = h2_p.tile([128, DCH, 512], BF16,
                                       name=f"h2_{psi}_{tgh}", tag="h2")
                        sq2 = sq2_p.tile([128, DCH, 512], BF16,
                                         name=f"sq2_{psi}_{tgh}", tag="sq2")
                        for dhalf in range(2):
                            psds = {}
                            for dq in range(4):
                                dc = dhalf * 4 + dq
                                psds[dc] = ps_dn.tile(
                                    [128, 512], F32,
                                    name=f"pd{psi}_{tgh}_{dc}", tag="ps_dn")
                            for m in range(HCH // 2):
                                for dq in range(4):
                                    dc = dhalf * 4 + dq
                                    nc.tensor.matmul(
                                        psds[dc][:],
                                        wd_s[:, 2 * m:2 * m + 2,
                                             ts(dc, 128)],
                                        ffn_t[:, 2 * m:2 * m + 2, fsl],
                                        start=(m == 0),
                                        stop=(m == HCH // 2 - 1),
                                        perf_mode=DR)
                            for dq in range(4):
                                dc = dhalf * 4 + dq
                                hu = otmp_p.tile([128, 512], F32,
                                                 name=f"hu{psi}_{tgh}_{dc}",
                                                 tag="hu")
                                nc.vector.tensor_scalar(
                                    out=hu[:], in0=psds[dc][:], scalar1=WSI,
                                    scalar2=bd_s[:, dc:dc + 1],
                                    op0=ALU.mult, op1=ALU.add)
                                nc.vector.tensor_tensor(
                                    out=h2[:, dc, :], in0=hu[:],
                                    in1=x1T[:, dc, tsl], op=ALU.add)
                                nc.scalar.activation(
                                    sq2[:, dc, :], h2[:, dc, :], AF.Square)
                        h2s[tgh] = (h2, sq2)

                    # LN2 for both groups after all down matmuls (stats of
                    # tgh=0 no longer stall the PE behind the evac chain)
                    for tgh in range(TGP):
                        tg = psi * TGP + tgh
                        tsl = ts(tg, 512)
                        h2, sq2 = h2s[tgh]
                        psm = ps_row2.tile([1, 512], F32,
                                           name=f"psm2_{psi}_{tgh}",
                                           tag="ps_row2")
                        for dc in range(DCH):
                            nc.tensor.matmul(psm[:], ones_col[:],
                                             h2[:, dc, :],
                                             start=(dc == 0),
                                             stop=(dc == DCH - 1))
                        pss = ps_row2.tile([1, 512], F32,
                                           name=f"pss2_{psi}_{tgh}",
                                           tag="ps_row2")
                        for dc in range(DCH):
                            nc.tensor.matmul(pss[:], ones_col[:],
                                             sq2[:, dc, :],
                                             start=(dc == 0),
                                             stop=(dc == DCH - 1))
                        mu = row2_p.tile([1, 512], F32,
                                         name=f"mu2_{psi}_{tgh}", tag="mu2")
                        nc.vector.tensor_scalar_mul(out=mu[:], in0=psm[:],
                                                    scalar1=1.0 / D)
                        ex2 = row2_p.tile([1, 512], F32,
                                          name=f"ex22_{psi}_{tgh}",
                                          tag="ex22")
                        nc.vector.tensor_scalar_mul(out=ex2[:], in0=pss[:],
                                                    scalar1=1.0 / D)
                        var = row2_p.tile([1, 512], F32,
                                          name=f"var2_{psi}_{tgh}",
                                          tag="var2")
                        nc.vector.tensor_tensor(out=var[:], in0=mu[:],
                                                in1=mu[:], op=ALU.mult)
                        nc.vector.tensor_tensor(out=var[:], in0=ex2[:],
                                                in1=var[:], op=ALU.subtract)
                        rstd = row2_p.tile([1, 512], F32,
                                           name=f"rstd2_{psi}_{tgh}",
                                           tag="rstd2")
                        nc.scalar.activation(rstd[:], var[:], AF.Sqrt,
                                             bias=epsr[:])
                        nc.vector.reciprocal(out=rstd[:], in_=rstd[:])
                        rstdb = row2_p.tile([1, 512], BF16,
                                            name=f"rstdb2_{psi}_{tgh}",
                                            tag="rstdb2")
                        nc.vector.tensor_copy(out=rstdb[:], in_=rstd[:])
                        nmr = row2_p.tile([1, 512], BF16,
                                          name=f"nmr2_{psi}_{tgh}",
                                          tag="nmr2")
                        nc.vector.scalar_tensor_tensor(
                            out=nmr[:], in0=mu[:], scalar=-1.0, in1=rstd[:],
                            op0=ALU.mult, op1=ALU.mult)
                        pbc_r = ps_bc2.tile([128, 512], F32,
                                            name=f"pbcr2_{psi}_{tgh}",
                                            tag="ps_bc2")
                        nc.tensor.matmul(pbc_r[:], ones_row[:], rstdb[:],
                                         start=True, stop=True)
                        pbc_m = ps_bc2.tile([128, 512], F32,
                                            name=f"pbcm2_{psi}_{tgh}",
                                            tag="ps_bc2")
                        nc.tensor.matmul(pbc_m[:], ones_row[:], nmr[:],
                                         start=True, stop=True)

                        o_t = ot_p.tile([128, DCH, 512], BF16,
                                        name=f"o_{psi}_{tgh}", tag="ot")
                        for dc in range(DCH):
                            u = otmp_p.tile([128, 512], F32,
                                            name=f"ou{psi}_{tgh}_{dc}",
                                            tag="ou")
                            nc.vector.scalar_tensor_tensor(
                                out=u[:], in0=h2[:, dc, :],
                                scalar=g2_s[:, dc:dc + 1], in1=pbc_r[:],
                                op0=ALU.mult, op1=ALU.mult)
                            nc.vector.scalar_tensor_tensor(
                                out=u[:], in0=pbc_m[:],
                                scalar=g2_s[:, dc:dc + 1], in1=u[:],
                                op0=ALU.mult, op1=ALU.add)
                            nc.scalar.activation(
                                o_t[:, dc, :], u[:], AF.Identity,
                                bias=b2_s[:, dc:dc + 1])
                        nc.sync.dma_start(out=out_v[:, :, tsl], in_=o_t[:])

    nc.compile()
    return nc, input_names


# ---------------------------------------------------------------------------
# Host-side wrapper
# ---------------------------------------------------------------------------

B, S, D_MODEL, D_FF = 4, 4096, 1024, 4096
FFN_H = int(2 * D_FF / 3)  # 2730

_cache = {}
LAST_RESULTS = None
BF16_NP = ml_dtypes.bfloat16
FP8_NP = ml_dtypes.float8_e4m3fn


def _get_program():
    if "prog" not in _cache:
        _cache["prog"] = build_program()
    return _cache["prog"]


def _prep_shared(Wqkv, bqkv, Wg, bg, Wu, bu, Wd, bd, g1, b1, g2, b2):
    f = np.float32

    def chunk_in(w):  # [1024, N] -> [128, 8, N] with d = c*128+p
        return np.ascontiguousarray(
            w.reshape(8, 128, -1).transpose(1, 0, 2)).astype(BF16_NP)

    Wqkv = np.asarray(Wqkv, f)
    sh = {}
    sh["wq"] = chunk_in(Wqkv[:, 0:1024])
    sh["wk"] = chunk_in(Wqkv[:, 1024:2048])
    sh["wv"] = chunk_in(Wqkv[:, 2048:3072])
    bqkv = np.asarray(bqkv, f)
    sh["bq_col"] = np.ascontiguousarray(
        bqkv[0:1024].reshape(8, 128).T).astype(f)
    sh["bk_row"] = bqkv[1024:2048].reshape(1, 1024).astype(BF16_NP)
    sh["bv_row"] = bqkv[2048:3072].reshape(1, 1024).astype(BF16_NP)

    wg_p = np.zeros((1024, H_PAD), f)
    wg_p[:, :FFN_H] = np.asarray(Wg, f)
    wu_p = np.zeros((1024, H_PAD), f)
    wu_p[:, :FFN_H] = np.asarray(Wu, f)

    def chunk_gu(w):  # [1024, 2816] -> [128, 11, 8, 256] fp8 x256
        w = (w * 256.0).reshape(8, 128, HBLK, 256)  # (c, p, hb, hsub)
        return np.ascontiguousarray(w.transpose(1, 2, 0, 3)).astype(FP8_NP)

    sh["wg"] = chunk_gu(wg_p)
    sh["wu"] = chunk_gu(wu_p)

    def col_pad(b, n):
        bp = np.zeros((n,), f)
        bp[:len(b)] = np.asarray(b, f)
        return np.ascontiguousarray(bp.reshape(n // 128, 128).T).astype(f)

    sh["bg_col"] = col_pad(np.asarray(bg, f), H_PAD)
    sh["bu_col"] = col_pad(np.asarray(bu, f), H_PAD)

    wd_p = np.zeros((H_PAD, 1024), f)
    wd_p[:FFN_H, :] = np.asarray(Wd, f)
    sh["wd"] = np.ascontiguousarray(
        (wd_p * 256.0).reshape(HCH, 128, 1024).transpose(1, 0, 2)
    ).astype(FP8_NP)
    sh["bd_col"] = col_pad(np.asarray(bd, f), 1024)
    sh["g1_col"] = col_pad(np.asarray(g1, f), 1024)
    sh["b1_col"] = col_pad(np.asarray(b1, f), 1024)
    sh["g2_col"] = col_pad(np.asarray(g2, f), 1024)
    sh["b2_col"] = col_pad(np.asarray(b2, f), 1024)
    return sh


def make_in_maps(x, Wqkv, bqkv, Wg, bg, Wu, bu, Wd, bd, g1, b1, g2, b2):
    x = np.asarray(x, np.float32)
    sh = _prep_shared(Wqkv, bqkv, Wg, bg, Wu, bu, Wd, bd, g1, b1, g2, b2)
    in_maps = []
    for c in range(8):
        b, h = c // 2, c % 2
        m = dict(sh)
        xs = x[b, h * 2048:(h + 1) * 2048]  # [2048, 1024]
        m["xh"] = np.ascontiguousarray(
            xs.reshape(2048, 8, 128).transpose(2, 1, 0)).astype(BF16_NP)
        in_maps.append(m)
    return in_maps


def kernel(x, Wqkv, bqkv, Wg, bg, Wu, bu, Wd, bd, g1, b1, g2, b2):
    global LAST_RESULTS
    from concourse import bass_utils

    nc, _names = _get_program()
    in_maps = make_in_maps(x, Wqkv, bqkv, Wg, bg, Wu, bu, Wd, bd,
                           g1, b1, g2, b2)
    res = bass_utils.run_bass_kernel_spmd(nc, in_maps, core_ids=list(range(8)))
    LAST_RESULTS = res
    out = np.empty((B, S, D_MODEL), np.float32)
    for c in range(8):
        b, h = c // 2, c % 2
        out[b, h * 2048:(h + 1) * 2048] = (
            res.results[c]["out"].astype(np.float32).T)
    return out


# revision 35
# speedup vs baseline: 1.4645x; 1.0293x over previous
"""Trainium2 Bass kernel for nn_Block_54219667145535 (linear-attention block).

v3: transposed (feature-major) pipeline, bf16 attention + fp8 DoubleRow FFN.

Sharding: 8 cores, 2 per batch (B=4). Each core computes k/v projections +
partial [D,D] kv state over its own 2048 tokens, pair-AllReduces the packed
(kv|ksum) buffer (single contiguous 16.4KB/partition descriptor), and hides
the collective under the q projection. Everything downstream stays in
[d-partition, token] orientation (no PE transposes, no x1 DRAM round-trip);
LN stats are ones-matmuls, per-token scales are rank-1 broadcast matmuls.
The SwiGLU FFN runs in fp8e4m3 with DoubleRow (K_eff=256), weights scaled by
256 on the host and descaled in the PSUM evacuation. Host pre-transposes x
and all weights and re-transposes the output.
"""

import os
import sys
from contextlib import ExitStack

import numpy as np


def _ensure_paths():
    for p in ("/opt/trn_rl_repo", "/root/.axon_site/_ro/trn_rl_repo"):
        if os.path.isdir(p) and p not in sys.path:
            sys.path.insert(0, p)
    try:
        import concourse.bass  # noqa: F401
    except ImportError as e:  # pragma: no cover
        raise ImportError(f"concourse not importable: {e}")


_ensure_paths()

import ml_dtypes  # noqa: E402

import concourse.bacc as bacc  # noqa: E402
import concourse.tile as tile  # noqa: E402
from concourse import mybir  # noqa: E402
from concourse.bass import ts  # noqa: E402

F32 = mybir.dt.float32
BF16 = mybir.dt.bfloat16
FP8 = mybir.dt.float8e4
AF = mybir.ActivationFunctionType
ALU = mybir.AluOpType
DR = mybir.MatmulPerfMode.DoubleRow

D = 1024
DCH = 8          # d chunks of 128
H_PAD = 2816
HCH = 22         # h chunks of 128
HBLK = 11        # h blocks of 256 (weight streaming granularity)
LN_EPS = 1e-5
ATTN_EPS = 1e-6
KV_W = DCH * D   # 8192 bf16 kv values per partition
PK_W = KV_W + DCH  # + 8 ksum values
WS = 256.0       # fp8 weight scale
WSI = 1.0 / WS


def build_program(T_OWN=2048, n_cores=8):
    """Per-core Bass/Tile program. Pair (2b, 2b+1) handles batch b."""
    NBLK = T_OWN // 512   # P1 token blocks
    NTG = T_OWN // 512    # P2/P3 token groups
    GROUPS = [[c, c + 1] for c in range(0, n_cores, 2)]

    nc = bacc.Bacc(
        "TRN2",
        target_bir_lowering=False,
        debug=False,
        enable_asserts=False,
        num_devices=8,
        num_swdge_queues=4,
    )

    # ---- I/O (host supplies pre-transposed / pre-chunked layouts) ----
    xh = nc.dram_tensor("xh", [128, DCH, T_OWN], BF16, kind="ExternalInput").ap()
    wk = nc.dram_tensor("wk", [128, DCH, D], BF16, kind="ExternalInput").ap()
    wv = nc.dram_tensor("wv", [128, DCH, D], BF16, kind="ExternalInput").ap()
    wq = nc.dram_tensor("wq", [128, DCH, D], BF16, kind="ExternalInput").ap()
    bk_row = nc.dram_tensor("bk_row", [1, D], BF16, kind="ExternalInput").ap()
    bv_row = nc.dram_tensor("bv_row", [1, D], BF16, kind="ExternalInput").ap()
    bq_col = nc.dram_tensor("bq_col", [128, DCH], F32, kind="ExternalInput").ap()
    # wg/wu: [p, hb, c, hsub] fp8 (x256); lhsT chunk = [:, hb, 2j:2j+2, hsl]
    wg = nc.dram_tensor("wg", [128, HBLK, DCH, 256], FP8,
                        kind="ExternalInput").ap()
    wu = nc.dram_tensor("wu", [128, HBLK, DCH, 256], FP8,
                        kind="ExternalInput").ap()
    bg_col = nc.dram_tensor("bg_col", [128, HCH], F32, kind="ExternalInput").ap()
    bu_col = nc.dram_tensor("bu_col", [128, HCH], F32, kind="ExternalInput").ap()
    # wd[p, hc, d] fp8 (x256)
    wd = nc.dram_tensor("wd", [128, HCH, D], FP8, kind="ExternalInput").ap()
    bd_col = nc.dram_tensor("bd_col", [128, DCH], F32, kind="ExternalInput").ap()
    g1_col = nc.dram_tensor("g1_col", [128, DCH], F32, kind="ExternalInput").ap()
    b1_col = nc.dram_tensor("b1_col", [128, DCH], F32, kind="ExternalInput").ap()
    g2_col = nc.dram_tensor("g2_col", [128, DCH], F32, kind="ExternalInput").ap()
    b2_col = nc.dram_tensor("b2_col", [128, DCH], F32, kind="ExternalInput").ap()
    out = nc.dram_tensor("out", [D, T_OWN], BF16, kind="ExternalOutput").ap()
    out_v = out.rearrange("(c p) t -> p c t", p=128)

    input_names = [
        "xh", "wk", "wv", "wq", "bk_row", "bv_row", "bq_col", "wg", "wu",
        "bg_col", "bu_col", "wd", "bd_col", "g1_col", "b1_col", "g2_col",
        "b2_col",
    ]

    def bcast_row(row_ap, parts=128):
        import concourse.bass as bass
        return bass.AP(
            tensor=row_ap.tensor,
            offset=row_ap.offset,
            ap=[[0, parts]] + [list(d) for d in row_ap.ap[1:]],
        )

    with tile.TileContext(nc) as tc, ExitStack() as top:
        dram = top.enter_context(tc.tile_pool(name="dram", bufs=1, space="DRAM"))
        kv_pack = dram.tile([128, PK_W], BF16, name="kv_pack")
        kv_out = dram.tile([128, PK_W], BF16, name="kv_out")
        x1_dram = dram.tile([128, DCH, T_OWN], BF16, name="x1_dram")

        consts = top.enter_context(tc.tile_pool(name="consts", bufs=1))
        ones_col = consts.tile([128, 1], BF16, name="ones_col")
        nc.vector.memset(ones_col[:], 1.0)
        ones_row = consts.tile([1, 128], BF16, name="ones_row")
        nc.vector.memset(ones_row[:], 1.0)
        one1 = consts.tile([1, 1], F32, name="one1")
        nc.vector.memset(one1[:], 1.0)
        epsr = consts.tile([1, 1], F32, name="epsr")
        nc.vector.memset(epsr[:], LN_EPS)
        epsb = consts.tile([128, 1], F32, name="epsb")
        nc.vector.memset(epsb[:], LN_EPS)
        bq_s = consts.tile([128, DCH], F32, name="bq_s")
        nc.sync.dma_start(out=bq_s[:], in_=bq_col)
        nbq_s = consts.tile([128, DCH], F32, name="nbq_s")
        nc.vector.tensor_scalar_mul(out=nbq_s[:], in0=bq_s[:], scalar1=-1.0)
        bg_s = consts.tile([128, HCH], F32, name="bg_s")
        nc.sync.dma_start(out=bg_s[:], in_=bg_col)
        bu_s = consts.tile([128, HCH], F32, name="bu_s")
        nc.sync.dma_start(out=bu_s[:], in_=bu_col)
        bd_s = consts.tile([128, DCH], F32, name="bd_s")
        nc.sync.dma_start(out=bd_s[:], in_=bd_col)
        g1_s = consts.tile([128, DCH], F32, name="g1_s")
        nc.sync.dma_start(out=g1_s[:], in_=g1_col)
        b1_s = consts.tile([128, DCH], F32, name="b1_s")
        nc.sync.dma_start(out=b1_s[:], in_=b1_col)
        g2_s = consts.tile([128, DCH], F32, name="g2_s")
        nc.sync.dma_start(out=g2_s[:], in_=g2_col)
        b2_s = consts.tile([128, DCH], F32, name="b2_s")
        nc.sync.dma_start(out=b2_s[:], in_=b2_col)

        # x1 (LN1 output): fp8 copy SBUF-resident for the FFN; the bf16 copy
        # round-trips through DRAM (written per-group in P2, re-read in P3).
        # These (and the FFN gate/up weight stream pool) sit at the bottom of
        # the stack so P3's weight prefetch never aliases P2 working pools.
        x1f8_p = top.enter_context(tc.tile_pool(name="x1f8p", bufs=1))
        x1f8 = x1f8_p.tile([128, DCH, T_OWN], FP8, name="x1f8")
        wgu_p = top.enter_context(tc.tile_pool(name="wgu", bufs=2))

        # x resident through P1+P2 (32KB/partition), freed before P3
        xres = ExitStack()
        xres_p = xres.enter_context(tc.tile_pool(name="xres", bufs=1))
        x_sb = xres_p.tile([128, DCH, T_OWN], BF16, name="x_sb")
        for blk in range(NBLK):
            nc.sync.dma_start(out=x_sb[:, :, ts(blk, 512)],
                              in_=xh[:, :, ts(blk, 512)])

        # kv+ksum accumulator, also the collective staging target
        p12 = ExitStack()
        accs = p12.enter_context(tc.tile_pool(name="accs", bufs=1))
        kv_sb = accs.tile([128, PK_W], BF16, name="kv_sb")
        kv_v = kv_sb[:][:, 0:KV_W].rearrange("p (c e) -> p c e", c=DCH)
        ks_v = kv_sb[:][:, KV_W:PK_W].rearrange("p (c e) -> p c e", c=DCH)

        # qp output pool opened before wqres to keep pool LIFO order
        qp_es = ExitStack()
        qp_p = qp_es.enter_context(tc.tile_pool(name="qp", bufs=1))
        qpT = qp_p.tile([128, DCH, T_OWN], BF16, name="qpT")

        # wq preloaded up-front so q-proj starts the instant P1 ends
        # (LIFO: closed right after the q-proj scope)
        wqres = ExitStack()
        wq_pool = wqres.enter_context(tc.tile_pool(name="wqres", bufs=1))
        wq_s = wq_pool.tile([128, DCH, D], BF16, name="wq_s")

        # ---------------- P1: k/v proj + kv/ksum over own tokens ----------
        with ExitStack() as p1:
            wkv_p = p1.enter_context(tc.tile_pool(name="wkv", bufs=1))
            c1_p = p1.enter_context(tc.tile_pool(name="c1", bufs=1))
            kpv_p = p1.enter_context(tc.tile_pool(name="kpv", bufs=1))
            tmp_p = p1.enter_context(tc.tile_pool(name="p1tmp", bufs=2))
            ksr_p = p1.enter_context(tc.tile_pool(name="ksr", bufs=1))
            ps_proj = p1.enter_context(
                tc.tile_pool(name="ps_proj", bufs=3, space="PSUM"))
            ps_kv = p1.enter_context(
                tc.tile_pool(name="ps_kv", bufs=3, space="PSUM"))
            ps_ks = p1.enter_context(
                tc.tile_pool(name="ps_ks", bufs=2, space="PSUM"))

            wk_s = wkv_p.tile([128, DCH, D], BF16, name="wk_s")
            wv_s = wkv_p.tile([128, DCH, D], BF16, name="wv_s")
            for half in range(2):
                nc.scalar.dma_start(out=wk_s[:, :, ts(half, 512)],
                                    in_=wk[:, :, ts(half, 512)])
            for half in range(2):
                nc.scalar.dma_start(out=wv_s[:, :, ts(half, 512)],
                                    in_=wv[:, :, ts(half, 512)])
            for half in range(2):
                nc.scalar.dma_start(out=wq_s[:, :, ts(half, 512)],
                                    in_=wq[:, :, ts(half, 512)])
            bkb = c1_p.tile([128, D], BF16, name="bkb")
            nc.sync.dma_start(out=bkb[:], in_=bcast_row(bk_row))
            bvb = c1_p.tile([128, D], BF16, name="bvb")
            nc.sync.dma_start(out=bvb[:], in_=bcast_row(bv_row))

            ks_ps = [ps_ks.tile([1, 512], F32, name=f"ksps{e}", tag="ps_ks")
                     for e in range(2)]

            for blk in range(NBLK):
                kp_blk = kpv_p.tile([128, 4, D], BF16, name=f"kp{blk}",
                                    tag="kp")
                v_blk = kpv_p.tile([128, 4, D], BF16, name=f"v{blk}", tag="v")

                for t4 in range(4):
                    tok0 = blk * 512 + t4 * 128
                    for which in range(2):  # 0 = k, 1 = v
                        w_s = wk_s if which == 0 else wv_s
                        for half in range(2):
                            gsl = ts(half, 512)
                            ps = ps_proj.tile(
                                [128, 512], F32,
                                name=f"pp{blk}_{t4}_{which}_{half}",
                                tag="ps_proj")
                            for dc in range(DCH):
                                nc.tensor.matmul(
                                    ps[:], x_sb[:, dc, tok0:tok0 + 128],
                                    w_s[:, dc, gsl],
                                    start=(dc == 0), stop=(dc == DCH - 1))
                            if which == 0:
                                # kp = elu(k+bk)+1 = exp(-r) + kb + r,
                                # r = relu(-kb), kb = k + bk
                                kb = tmp_p.tile([128, 512], F32,
                                                name=f"kb{blk}_{t4}_{half}",
                                                tag="kb")
                                nc.vector.tensor_tensor(
                                    out=kb[:], in0=ps[:], in1=bkb[:, gsl],
                                    op=ALU.add)
                                r = tmp_p.tile([128, 512], F32,
                                               name=f"r{blk}_{t4}_{half}",
                                               tag="r")
                                nc.scalar.activation(r[:], kb[:], AF.Relu,
                                                     scale=-1.0)
                                e = tmp_p.tile([128, 512], F32,
                                               name=f"e{blk}_{t4}_{half}",
                                               tag="e")
                                nc.scalar.activation(e[:], r[:], AF.Exp,
                                                     scale=-1.0)
                                nc.vector.tensor_tensor(
                                    out=kb[:], in0=kb[:], in1=r[:], op=ALU.add)
                                nc.vector.tensor_tensor(
                                    out=kp_blk[:, t4, gsl], in0=kb[:],
                                    in1=e[:], op=ALU.add)
                            else:
                                nc.vector.tensor_tensor(
                                    out=v_blk[:, t4, gsl], in0=ps[:],
                                    in1=bvb[:, gsl], op=ALU.add)

                    # ksum += ones^T @ kp for this t4 (both e halves)
                    for ec in range(2):
                        nc.tensor.matmul(
                            ks_ps[ec][:], ones_col[:],
                            kp_blk[:, t4, ts(ec, 512)],
                            start=(blk == 0 and t4 == 0),
                            stop=(blk == NBLK - 1 and t4 == 3))

                # kv[dc, e-half] += kp_blk^T @ v_blk
                for dc in range(DCH):
                    dsl = ts(dc, 128)
                    for ec in range(2):
                        esl = ts(ec, 512)
                        pkv = ps_kv.tile([128, 512], F32,
                                         name=f"pkv{blk}_{dc}_{ec}",
                                         tag="ps_kv")
                        for t4 in range(4):
                            nc.tensor.matmul(
                                pkv[:], kp_blk[:, t4, dsl],
                                v_blk[:, t4, esl],
                                start=(t4 == 0), stop=(t4 == 3))
                        if blk == 0:
                            nc.vector.tensor_copy(
                                out=kv_v[:, dc, esl], in_=pkv[:])
                        else:
                            nc.vector.tensor_tensor(
                                out=kv_v[:, dc, esl], in0=kv_v[:, dc, esl],
                                in1=pkv[:], op=ALU.add)

            # ksum rows [1,1024] -> per-partition cols kv_sb[:, KV_W:]
            ks_row = ksr_p.tile([1, D], F32, name="ks_row")
            for ec in range(2):
                nc.scalar.copy(out=ks_row[:, ts(ec, 512)], in_=ks_ps[ec][:])
            for dc in range(DCH):
                ptk = ps_proj.tile([128, 1], F32, name=f"ptk{dc}",
                                   tag="ps_proj")
                nc.tensor.transpose(ptk[:], ks_row[:, ts(dc, 128)],
                                    one1[:])
                nc.scalar.copy(out=ks_v[:, dc, :], in_=ptk[:])

        # ---- pair AllReduce of packed (kv | ksum), single descriptor ----
        nc.sync.dma_start(out=kv_pack[:], in_=kv_sb[:])
        nc.gpsimd.collective_compute(
            "AllReduce", ALU.add,
            ins=[kv_pack[:]], outs=[kv_out[:]], replica_groups=GROUPS)
        nc.gpsimd.dma_start(out=kv_sb[:], in_=kv_out[:])

        # ---------------- P2: q proj (hides collective), then attn + LN1 --
        with ExitStack() as p2:
            with ExitStack() as qsc:
                qtmp_p = qsc.enter_context(tc.tile_pool(name="qtmp", bufs=3))
                ps_q = qsc.enter_context(
                    tc.tile_pool(name="ps_q", bufs=3, space="PSUM"))
                for tg in range(NTG):
                    tsl = ts(tg, 512)
                    for qc in range(DCH):
                        ps = ps_q.tile([128, 512], F32, name=f"pq{tg}_{qc}",
                                       tag="ps_q")
                        for dc in range(DCH):
                            nc.tensor.matmul(
                                ps[:], wq_s[:, dc, ts(qc, 128)],
                                x_sb[:, dc, tsl],
                                start=(dc == 0), stop=(dc == DCH - 1))
                        # qp = elu(q+bq)+1; per-partition bias
                        r = qtmp_p.tile([128, 512], F32, name=f"qr{tg}_{qc}",
                                        tag="qr")
                        nc.scalar.activation(r[:], ps[:], AF.Relu,
                                             scale=-1.0,
                                             bias=nbq_s[:, qc:qc + 1])
                        e = qtmp_p.tile([128, 512], F32, name=f"qe{tg}_{qc}",
                                        tag="qe")
                        nc.scalar.activation(e[:], r[:], AF.Exp, scale=-1.0)
                        t = qtmp_p.tile([128, 512], F32, name=f"qt{tg}_{qc}",
                                        tag="qt")
                        nc.vector.scalar_tensor_tensor(
                            out=t[:], in0=ps[:], scalar=bq_s[:, qc:qc + 1],
                            in1=r[:], op0=ALU.add, op1=ALU.add)
                        nc.vector.tensor_tensor(
                            out=qpT[:, qc, tsl], in0=t[:], in1=e[:],
                            op=ALU.add)
            wqres.close()

            with ExitStack() as asc:
                h1_p = asc.enter_context(tc.tile_pool(name="h1", bufs=2))
                sq_p = asc.enter_context(tc.tile_pool(name="sq", bufs=2))
                row_p = asc.enter_context(tc.tile_pool(name="rows", bufs=2))
                atmp_p = asc.enter_context(tc.tile_pool(name="atmp", bufs=3))
                lnt_p = asc.enter_context(tc.tile_pool(name="lnt", bufs=1))
                dbc_p = asc.enter_context(tc.tile_pool(name="dbc", bufs=2))
                x1t_p = asc.enter_context(tc.tile_pool(name="x1t", bufs=2))
                ps_row = asc.enter_context(
                    tc.tile_pool(name="ps_row", bufs=3, space="PSUM"))
                ps_num = asc.enter_context(
                    tc.tile_pool(name="ps_num", bufs=2, space="PSUM"))
                ps_bc = asc.enter_context(
                    tc.tile_pool(name="ps_bc", bufs=3, space="PSUM"))

                hs = {}

                def stage_a(tg):
                    tsl = ts(tg, 512)
                    # den row = ksum^T qp; broadcast, then +eps/recip at
                    # full width (single-partition row math is ~6x slower)
                    pdn = ps_row.tile([1, 512], F32, name=f"pdn{tg}",
                                      tag="ps_row")
                    for dc in range(DCH):
                        nc.tensor.matmul(
                            pdn[:], ks_v[:, dc, :], qpT[:, dc, tsl],
                            start=(dc == 0), stop=(dc == DCH - 1))
                    dnb = row_p.tile([1, 512], BF16, name=f"dnb{tg}",
                                     tag="dnb")
                    nc.scalar.copy(out=dnb[:], in_=pdn[:])
                    pbc_d = ps_bc.tile([128, 512], F32, name=f"pbcd{tg}",
                                       tag="ps_bc")
                    nc.tensor.matmul(pbc_d[:], ones_row[:], dnb[:],
                                     start=True, stop=True)
                    de = dbc_p.tile([128, 512], F32, name=f"de{tg}",
                                    tag="de")
                    nc.vector.tensor_scalar_add(
                        out=de[:], in0=pbc_d[:], scalar1=ATTN_EPS)
                    dbc = dbc_p.tile([128, 512], F32, name=f"dbc{tg}",
                                     tag="dbc")
                    nc.vector.reciprocal(out=dbc[:], in_=de[:])

                    h1 = h1_p.tile([128, DCH, 512], BF16, name=f"h1_{tg}",
                                   tag="h1")
                    sq = sq_p.tile([128, DCH, 512], BF16, name=f"sq_{tg}",
                                   tag="sq")
                    for ec in range(DCH):
                        pn = ps_num.tile([128, 512], F32,
                                         name=f"pn{tg}_{ec}", tag="ps_num")
                        for dc in range(DCH):
                            nc.tensor.matmul(
                                pn[:], kv_v[:, dc, ts(ec, 128)],
                                qpT[:, dc, tsl],
                                start=(dc == 0), stop=(dc == DCH - 1))
                        at = atmp_p.tile([128, 512], BF16,
                                         name=f"at{tg}_{ec}", tag="at")
                        nc.vector.tensor_tensor(
                            out=at[:], in0=pn[:], in1=dbc[:], op=ALU.mult)
                        nc.gpsimd.tensor_tensor(
                            out=h1[:, ec, :], in0=at[:],
                            in1=x_sb[:, ec, tsl], op=ALU.add)
                        nc.gpsimd.tensor_tensor(
                            out=sq[:, ec, :], in0=h1[:, ec, :],
                            in1=h1[:, ec, :], op=ALU.mult)
                    hs[tg] = (h1, sq)

                def stage_b(tg):
                    tsl = ts(tg, 512)
                    h1, sq = hs[tg]
                    psm = ps_row.tile([1, 512], F32, name=f"psm{tg}",
                                      tag="ps_row")
                    for dc in range(DCH):
                        nc.tensor.matmul(psm[:], ones_col[:], h1[:, dc, :],
                                         start=(dc == 0),
                                         stop=(dc == DCH - 1))
                    pss = ps_row.tile([1, 512], F32, name=f"pss{tg}",
                                      tag="ps_row")
                    for dc in range(DCH):
                        nc.tensor.matmul(pss[:], ones_col[:], sq[:, dc, :],
                                         start=(dc == 0),
                                         stop=(dc == DCH - 1))
                    srow = row_p.tile([1, 512], BF16, name=f"sr{tg}",
                                      tag="srow")
                    nc.scalar.copy(out=srow[:], in_=psm[:])
                    qrow = row_p.tile([1, 512], BF16, name=f"qr{tg}",
                                      tag="qrow")
                    nc.scalar.copy(out=qrow[:], in_=pss[:])
                    sb = ps_bc.tile([128, 512], F32, name=f"sb{tg}",
                                    tag="ps_bc")
                    nc.tensor.matmul(sb[:], ones_row[:], srow[:],
                                     start=True, stop=True)
                    qb = ps_bc.tile([128, 512], F32, name=f"qb{tg}",
                                    tag="ps_bc")
                    nc.tensor.matmul(qb[:], ones_row[:], qrow[:],
                                     start=True, stop=True)
                    # mu_b = sb/D ; var = qb/D - mu_b^2 ; rstd_b
                    mu_b = lnt_p.tile([128, 512], F32, name=f"mub{tg}",
                                      tag="mu_b")
                    nc.vector.tensor_scalar_mul(out=mu_b[:], in0=sb[:],
                                                scalar1=1.0 / D)
                    m2 = lnt_p.tile([128, 512], F32, name=f"m2{tg}",
                                    tag="m2")
                    nc.vector.tensor_tensor(out=m2[:], in0=mu_b[:],
                                            in1=mu_b[:], op=ALU.mult)
                    vr = lnt_p.tile([128, 512], F32, name=f"vr{tg}",
                                    tag="vr")
                    nc.vector.scalar_tensor_tensor(
                        out=vr[:], in0=qb[:], scalar=1.0 / D, in1=m2[:],
                        op0=ALU.mult, op1=ALU.subtract)
                    sd = lnt_p.tile([128, 512], F32, name=f"sd{tg}",
                                    tag="sd")
                    nc.scalar.activation(sd[:], vr[:], AF.Sqrt,
                                         bias=epsb[:])
                    rstd_b = lnt_p.tile([128, 512], F32, name=f"rsb{tg}",
                                        tag="rstd_b")
                    nc.vector.reciprocal(out=rstd_b[:], in_=sd[:])

                    # x1 = ((h1 - mu_b) * rstd_b) * g1 + b1  (bf16 + fp8)
                    x1t = x1t_p.tile([128, DCH, 512], BF16,
                                     name=f"x1t{tg}", tag="x1t")
                    for dc in range(DCH):
                        v1 = atmp_p.tile([128, 512], F32,
                                         name=f"v1{tg}_{dc}", tag="v1")
                        nc.vector.tensor_tensor(
                            out=v1[:], in0=h1[:, dc, :], in1=mu_b[:],
                            op=ALU.subtract)
                        v2 = atmp_p.tile([128, 512], F32,
                                         name=f"v2{tg}_{dc}", tag="v2")
                        nc.vector.scalar_tensor_tensor(
                            out=v2[:], in0=v1[:],
                            scalar=g1_s[:, dc:dc + 1], in1=rstd_b[:],
                            op0=ALU.mult, op1=ALU.mult)
                        nc.scalar.activation(
                            x1t[:, dc, :], v2[:], AF.Identity,
                            bias=b1_s[:, dc:dc + 1])
                        nc.scalar.activation(
                            x1f8[:, dc, tsl], v2[:], AF.Identity,
                            bias=b1_s[:, dc:dc + 1])
                    nc.sync.dma_start(out=x1_dram[:][:, :, tsl],
                                      in_=x1t[:])

                # software pipeline: stats of tg overlap num of tg+1
                stage_a(0)
                for tg in range(1, NTG):
                    stage_a(tg)
                    stage_b(tg - 1)
                stage_b(NTG - 1)

        qp_es.close()  # qp dead after attn
        p12.close()  # kv/ksum accumulators dead after P2
        xres.close()  # x dead after P2

        # ---------------- P3: fp8 DoubleRow SwiGLU FFN + LN2 --------------
        NPASS = 2
        TGP = NTG // NPASS
        with ExitStack() as p3:
            ffn_p = p3.enter_context(tc.tile_pool(name="ffn", bufs=1))
            wd_p = p3.enter_context(tc.tile_pool(name="wdp", bufs=1))
            x1r_p = p3.enter_context(tc.tile_pool(name="x1r", bufs=2))

            # wd resident for all of P3 (22KB/partition fp8)
            wd_s = wd_p.tile([128, HCH, D], FP8, name="wd_s")
            for hh in range(2):
                nc.sync.dma_start(out=wd_s[:, ts(hh, HCH // 2), :],
                                  in_=wd[:, ts(hh, HCH // 2), :])

            for psi in range(NPASS):
                ffn_t = ffn_p.tile([128, HCH, TGP * 512], FP8,
                                   name=f"ffn{psi}", tag="ffn")

                with ExitStack() as gsc:
                    ps_g = gsc.enter_context(
                        tc.tile_pool(name="ps_g", bufs=2, space="PSUM"))
                    ps_u = gsc.enter_context(
                        tc.tile_pool(name="ps_u", bufs=2, space="PSUM"))
                    sg_p = gsc.enter_context(tc.tile_pool(name="sg", bufs=3))
                    ut_p = gsc.enter_context(tc.tile_pool(name="ut", bufs=3))
                    for hb in range(HBLK):
                        wg_t = wgu_p.tile([128, DCH, 256], FP8,
                                          name=f"wg{psi}_{hb}", tag="wg")
                        nc.scalar.dma_start(out=wg_t[:], in_=wg[:, hb])
                        wu_t = wgu_p.tile([128, DCH, 256], FP8,
                                          name=f"wu{psi}_{hb}", tag="wu")
                        nc.scalar.dma_start(out=wu_t[:], in_=wu[:, hb])
                        for hl in range(2):
                            hc = hb * 2 + hl
                            hsl = ts(hl, 128)
                            for tgh in range(TGP):
                                tg = psi * TGP + tgh
                                tsl = ts(tg, 512)
                                fsl = ts(tgh, 512)
                                psg = ps_g.tile([128, 512], F32,
                                                name=f"pg{psi}_{hc}_{tgh}",
                                                tag="ps_g")
                                for jp in range(DCH // 2):
                                    nc.tensor.matmul(
                                        psg[:],
                                        wg_t[:, 2 * jp:2 * jp + 2, hsl],
                                        x1f8[:, 2 * jp:2 * jp + 2, tsl],
                                        start=(jp == 0),
                                        stop=(jp == DCH // 2 - 1),
                                        perf_mode=DR)
                                psu = ps_u.tile([128, 512], F32,
                                                name=f"pu{psi}_{hc}_{tgh}",
                                                tag="ps_u")
                                for jp in range(DCH // 2):
                                    nc.tensor.matmul(
                                        psu[:],
                                        wu_t[:, 2 * jp:2 * jp + 2, hsl],
                                        x1f8[:, 2 * jp:2 * jp + 2, tsl],
                                        start=(jp == 0),
                                        stop=(jp == DCH // 2 - 1),
                                        perf_mode=DR)
                                sg = sg_p.tile([128, 512], F32,
                                               name=f"sg{psi}_{hc}_{tgh}",
                                               tag="sg")
                                nc.scalar.activation(
                                    sg[:], psg[:], AF.Silu,
                                    bias=bg_s[:, hc:hc + 1], scale=WSI)
                                ut = ut_p.tile([128, 512], F32,
                                               name=f"ut{psi}_{hc}_{tgh}",
                                               tag="ut")
                                nc.vector.tensor_scalar(
                                    out=ut[:], in0=psu[:], scalar1=WSI,
                                    scalar2=bu_s[:, hc:hc + 1],
                                    op0=ALU.mult, op1=ALU.add)
                                nc.vector.tensor_tensor(
                                    out=ffn_t[:, hc, fsl], in0=ut[:],
                                    in1=sg[:], op=ALU.mult)

                with ExitStack() as dsc:
                    h2_p = dsc.enter_context(tc.tile_pool(name="h2", bufs=2))
                    sq2_p = dsc.enter_context(tc.tile_pool(name="sq2",
                                                           bufs=2))
                    row2_p = dsc.enter_context(tc.tile_pool(name="rows2",
                                                            bufs=2))
                    ln2_p = dsc.enter_context(tc.tile_pool(name="ln2t",
                                                           bufs=1))
                    ot_p = dsc.enter_context(tc.tile_pool(name="otp",
                                                          bufs=2))
                    otmp_p = dsc.enter_context(tc.tile_pool(name="otmp",
                                                            bufs=2))
                    ps_dn = dsc.enter_context(
                        tc.tile_pool(name="ps_dn", bufs=4, space="PSUM"))
                    ps_row2 = dsc.enter_context(
                        tc.tile_pool(name="ps_row2", bufs=2, space="PSUM"))
                    ps_bc2 = dsc.enter_context(
                        tc.tile_pool(name="ps_bc2", bufs=2, space="PSUM"))

                    h2s = {}
                    for tgh in range(TGP):
                        tg = psi * TGP + tgh
                        tsl = ts(tg, 512)
                        fsl = ts(tgh, 512)
                        x1r = x1r_p.tile([128, DCH, 512], BF16,
                                         name=f"x1r{psi}_{tgh}", tag="x1r")
                        nc.gpsimd.dma_start(out=x1r[:],
                                            in_=x1_dram[:][:, :, tsl])
                        h2 = h2_p.tile([128, DCH, 512], BF16,
                                       name=f"h2_{psi}_{tgh}", tag="h2")
                        sq2 = sq2_p.tile([128, DCH, 512], BF16,
                                         name=f"sq2_{psi}_{tgh}", tag="sq2")
                        for dhalf in range(2):
                            psds = {}
                            for dq in range(4):
                                dc = dhalf * 4 + dq
                                psds[dc] = ps_dn.tile(
                                    [128, 512], F32,
                                    name=f"pd{psi}_{tgh}_{dc}", tag="ps_dn")
                            for m in range(HCH // 2):
                                for dq in range(4):
                                    dc = dhalf * 4 + dq
                                    nc.tensor.matmul(
                                        psds[dc][:],
                                        wd_s[:, 2 * m:2 * m + 2,
                                             ts(dc, 128)],
                                        ffn_t[:, 2 * m:2 * m + 2, fsl],
                                        start=(m == 0),
                                        stop=(m == HCH // 2 - 1),
                                        perf_mode=DR)
                            for dq in range(4):
                                dc = dhalf * 4 + dq
                                hu = otmp_p.tile([128, 512], BF16,
                                                 name=f"hu{psi}_{tgh}_{dc}",
                                                 tag="hu")
                                nc.vector.tensor_scalar(
                                    out=hu[:], in0=psds[dc][:], scalar1=WSI,
                                    scalar2=bd_s[:, dc:dc + 1],
                                    op0=ALU.mult, op1=ALU.add)
                                nc.gpsimd.tensor_tensor(
                                    out=h2[:, dc, :], in0=hu[:],
                                    in1=x1r[:, dc, :], op=ALU.add)
                                nc.gpsimd.tensor_tensor(
                                    out=sq2[:, dc, :], in0=h2[:, dc, :],
                                    in1=h2[:, dc, :], op=ALU.mult)
                        h2s[tgh] = (h2, sq2)

                    # LN2 for both groups after all down matmuls
                    for tgh in range(TGP):
                        tg = psi * TGP + tgh
                        tsl = ts(tg, 512)
                        h2, sq2 = h2s[tgh]
                        psm = ps_row2.tile([1, 512], F32,
                                           name=f"psm2_{psi}_{tgh}",
                                           tag="ps_row2")
                        for dc in range(DCH):
                            nc.tensor.matmul(psm[:], ones_col[:],
                                             h2[:, dc, :],
                                             start=(dc == 0),
                                             stop=(dc == DCH - 1))
                        pss = ps_row2.tile([1, 512], F32,
                                           name=f"pss2_{psi}_{tgh}",
                                           tag="ps_row2")
                        for dc in range(DCH):
                            nc.tensor.matmul(pss[:], ones_col[:],
                                             sq2[:, dc, :],
                                             start=(dc == 0),
                                             stop=(dc == DCH - 1))
                        srow = row2_p.tile([1, 512], BF16,
                                           name=f"sr2{psi}_{tgh}",
                                           tag="srow2")
                        nc.scalar.copy(out=srow[:], in_=psm[:])
                        qrow = row2_p.tile([1, 512], BF16,
                                           name=f"qr2{psi}_{tgh}",
                                           tag="qrow2")
                        nc.scalar.copy(out=qrow[:], in_=pss[:])
                        sb = ps_bc2.tile([128, 512], F32,
                                         name=f"sb2{psi}_{tgh}",
                                         tag="ps_bc2")
                        nc.tensor.matmul(sb[:], ones_row[:], srow[:],
                                         start=True, stop=True)
                        qb = ps_bc2.tile([128, 512], F32,
                                         name=f"qb2{psi}_{tgh}",
                                         tag="ps_bc2")
                        nc.tensor.matmul(qb[:], ones_row[:], qrow[:],
                                         start=True, stop=True)
                        mu_b = ln2_p.tile([128, 512], F32,
                                          name=f"mub2{psi}_{tgh}",
                                          tag="mu_b2")
                        nc.vector.tensor_scalar_mul(out=mu_b[:], in0=sb[:],
                                                    scalar1=1.0 / D)
                        m2 = ln2_p.tile([128, 512], F32,
                                        name=f"m22{psi}_{tgh}", tag="m22")
                        nc.vector.tensor_tensor(out=m2[:], in0=mu_b[:],
                                                in1=mu_b[:], op=ALU.mult)
                        vr = ln2_p.tile([128, 512], F32,
                                        name=f"vr2{psi}_{tgh}", tag="vr2")
                        nc.vector.scalar_tensor_tensor(
                            out=vr[:], in0=qb[:], scalar=1.0 / D, in1=m2[:],
                            op0=ALU.mult, op1=ALU.subtract)
                        sd = ln2_p.tile([128, 512], F32,
                                        name=f"sd2{psi}_{tgh}", tag="sd2")
                        nc.scalar.activation(sd[:], vr[:], AF.Sqrt,
                                             bias=epsb[:])
                        rstd_b = ln2_p.tile([128, 512], F32,
                                            name=f"rsb2{psi}_{tgh}",
                                            tag="rstd_b2")
                        nc.vector.reciprocal(out=rstd_b[:], in_=sd[:])

                        o_t = ot_p.tile([128, DCH, 512], BF16,
                                        name=f"o_{psi}_{tgh}", tag="ot")
                        for dc in range(DCH):
                            v1 = otmp_p.tile([128, 512], F32,
                                             name=f"ov{psi}_{tgh}_{dc}",
                                             tag="ov")
                            nc.vector.tensor_tensor(
                                out=v1[:], in0=h2[:, dc, :], in1=mu_b[:],
                                op=ALU.subtract)
                            v2 = otmp_p.tile([128, 512], F32,
                                             name=f"ow{psi}_{tgh}_{dc}",
                                             tag="ow")
                            nc.vector.scalar_tensor_tensor(
                                out=v2[:], in0=v1[:],
                                scalar=g2_s[:, dc:dc + 1], in1=rstd_b[:],
                                op0=ALU.mult, op1=ALU.mult)
                            nc.scalar.activation(
                                o_t[:, dc, :], v2[:], AF.Identity,
                                bias=b2_s[:, dc:dc + 1])
                        nc.sync.dma_start(out=out_v[:, :, tsl], in_=o_t[:])

    nc.compile()
    return nc, input_names


# ---------------------------------------------------------------------------
# Host-side wrapper
# ---------------------------------------------------------------------------

B, S, D_MODEL, D_FF = 4, 4096, 1024, 4096
FFN_H = int(2 * D_FF / 3)  # 2730

_cache = {}
LAST_RESULTS = None
BF16_NP = ml_dtypes.bfloat16
FP8_NP = ml_dtypes.float8_e4m3fn


def _get_program():
    if "prog" not in _cache:
        _cache["prog"] = build_program()
    return _cache["prog"]


def _prep_shared(Wqkv, bqkv, Wg, bg, Wu, bu, Wd, bd, g1, b1, g2, b2):
    f = np.float32

    def chunk_in(w):  # [1024, N] -> [128, 8, N] with d = c*128+p
        return np.ascontiguousarray(
            w.reshape(8, 128, -1).transpose(1, 0, 2)).astype(BF16_NP)

    Wqkv = np.asarray(Wqkv, f)
    sh = {}
    sh["wq"] = chunk_in(Wqkv[:, 0:1024])
    sh["wk"] = chunk_in(Wqkv[:, 1024:2048])
    sh["wv"] = chunk_in(Wqkv[:, 2048:3072])
    bqkv = np.asarray(bqkv, f)
    sh["bq_col"] = np.ascontiguousarray(
        bqkv[0:1024].reshape(8, 128).T).astype(f)
    sh["bk_row"] = bqkv[1024:2048].reshape(1, 1024).astype(BF16_NP)
    sh["bv_row"] = bqkv[2048:3072].reshape(1, 1024).astype(BF16_NP)

    wg_p = np.zeros((1024, H_PAD), f)
    wg_p[:, :FFN_H] = np.asarray(Wg, f)
    wu_p = np.zeros((1024, H_PAD), f)
    wu_p[:, :FFN_H] = np.asarray(Wu, f)

    def chunk_gu(w):  # [1024, 2816] -> [128, 11, 8, 256] fp8 x256
        w = (w * 256.0).reshape(8, 128, HBLK, 256)  # (c, p, hb, hsub)
        return np.ascontiguousarray(w.transpose(1, 2, 0, 3)).astype(FP8_NP)

    sh["wg"] = chunk_gu(wg_p)
    sh["wu"] = chunk_gu(wu_p)

    def col_pad(b, n):
        bp = np.zeros((n,), f)
        bp[:len(b)] = np.asarray(b, f)
        return np.ascontiguousarray(bp.reshape(n // 128, 128).T).astype(f)

    sh["bg_col"] = col_pad(np.asarray(bg, f), H_PAD)
    sh["bu_col"] = col_pad(np.asarray(bu, f), H_PAD)

    wd_p = np.zeros((H_PAD, 1024), f)
    wd_p[:FFN_H, :] = np.asarray(Wd, f)
    sh["wd"] = np.ascontiguousarray(
        (wd_p * 256.0).reshape(HCH, 128, 1024).transpose(1, 0, 2)
    ).astype(FP8_NP)
    sh["bd_col"] = col_pad(np.asarray(bd, f), 1024)
    sh["g1_col"] = col_pad(np.asarray(g1, f), 1024)
    sh["b1_col"] = col_pad(np.asarray(b1, f), 1024)
    sh["g2_col"] = col_pad(np.asarray(g2, f), 1024)
    sh["b2_col"] = col_pad(np.asarray(b2, f), 1024)
    return sh


def make_in_maps(x, Wqkv, bqkv, Wg, bg, Wu, bu, Wd, bd, g1, b1, g2, b2):
    x = np.asarray(x, np.float32)
    sh = _prep_shared(Wqkv, bqkv, Wg, bg, Wu, bu, Wd, bd, g1, b1, g2, b2)
    in_maps = []
    for c in range(8):
        b, h = c // 2, c % 2
        m = dict(sh)
        xs = x[b, h * 2048:(h + 1) * 2048]  # [2048, 1024]
        m["xh"] = np.ascontiguousarray(
            xs.reshape(2048, 8, 128).transpose(2, 1, 0)).astype(BF16_NP)
        in_maps.append(m)
    return in_maps


def kernel(x, Wqkv, bqkv, Wg, bg, Wu, bu, Wd, bd, g1, b1, g2, b2):
    global LAST_RESULTS
    from concourse import bass_utils

    nc, _names = _get_program()
    in_maps = make_in_maps(x, Wqkv, bqkv, Wg, bg, Wu, bu, Wd, bd,
                           g1, b1, g2, b2)
    res = bass_utils.run_bass_kernel_spmd(nc, in_maps, core_ids=list(range(8)))
    LAST_RESULTS = res
    out = np.empty((B, S, D_MODEL), np.float32)
    for c in range(8):
        b, h = c // 2, c % 2
        out[b, h * 2048:(h + 1) * 2048] = (
            res.results[c]["out"].astype(np.float32).T)
    return out
